# revision 1
# baseline (speedup 1.0000x reference)
"""NeuralHMM forward log-likelihood on 8 Trainium2 NeuronCores.

Strategy (data-parallel over time slabs, associative chunk combine):
  - Core k owns time slab t in [256k, 256(k+1)) for ALL batch elements,
    for both phases (no cross-core data except the final AllGather).
  - Phase 1 (parallel over (b,t)): transition MLP -> E = exp(logits) (bf16,
    row-unnormalized transition kernel) staged to DRAM in [b][t][i*64+j]
    layout; per-row sums R -> invR and obs-model log-probs -> per-t
    normalizer n(b,t) (column mean over states) and ehat = exp(obs_lp - n),
    both stored transposed [b][state][t] so the chain reads per-t columns.
  - Phase 2: linear-domain operator chains. Per (b, half-slab) chunk of 128
    steps:  M = prod_t D(s_t) E_t^T,  s_t = ehat_t * invR_{t+1} (last step
    of a chunk uses ehat only; chunk init is D(invR_lo)).  8 chunks per
    core run interleaved; the 8 per-step scale ops are fused into one DVE
    instruction over a single packed PSUM bank.  A ones-column appended to
    each E gives column masses for free; periodic single-scalar renorm
    keeps fp32 range with exact log accounting.
  - AllGather of per-chunk operators + scalar corrections, then every core
    redundantly combines: ll_b = log(1^T M_15 ... M_0 a0) + sum_t n(b,t)
    + sum(renorm logs) - log(S).
Weight-only reshapes/transposes are precomputed on host in kernel().
"""

import math
import os
import sys

import numpy as np

BUILD_STAGE = int(os.environ.get("NHMM_STAGE", "4"))  # 1=p1, 2=+chains, 3=+cc, 4=all
P1_STAGE = int(os.environ.get("NHMM_P1", "4"))  # 1=loads, 2=+mlp, 3=+logits, 4=all
DEBUG = os.environ.get("NHMM_DEBUG", "0") == "1"

sys.path.insert(0, "/opt/trn_rl_repo")

import ml_dtypes  # noqa: E402

import concourse.bass as bass  # noqa: E402
import concourse.tile as tile  # noqa: E402
from concourse import bacc, mybir  # noqa: E402
from concourse.bass_utils import run_bass_kernel_spmd  # noqa: E402
from concourse.masks import make_identity  # noqa: E402

F32 = mybir.dt.float32
BF16 = mybir.dt.bfloat16
AF = mybir.ActivationFunctionType
AX = mybir.AxisListType
ALU = mybir.AluOpType

B, T, D = 4, 2048, 80
S, H, C = 64, 256, 128
NCORES = 8
SLAB = T // NCORES        # 256 timesteps per core
NTILE = 128               # phase-1 tile width
NT_TILES = SLAB // NTILE  # 2
NCHAIN = 8                # interleaved operator chains per core
CHUNK = SLAB // 2         # 128 steps per chain
SEG = 32                  # steps per DMA segment
NSEG = CHUNK // SEG       # 4 segments per chunk
RENORM_K = 16
NSLOT = CHUNK // RENORM_K  # 8 renorm slots per chain
LSLICE = 512

CONTRIB = NCHAIN * S * S + B + NCHAIN + B * S
OFF_M = 0
OFF_NU = NCHAIN * S * S
OFF_LAM = OFF_NU + B
OFF_A0 = OFF_LAM + NCHAIN


def build_program():
    nc = bacc.Bacc(
        "TRN2",
        target_bir_lowering=False,
        debug=False,
        enable_asserts=False,
        num_devices=NCORES,
    )

    def din(name, shape, dtype=F32):
        return nc.dram_tensor(name, list(shape), dtype, kind="ExternalInput").ap()

    io = {}
    io["obs"] = din("obs_slab", (B, SLAB, D))
    io["ctx"] = din("ctx_slab", (B, SLAB, C), BF16)
    io["alpha"] = din("alpha_blend", (S, NCHAIN))
    io["tw1cT"] = din("tw1cT", (C, H), BF16)
    io["tb1p"] = din("tb1p", (H,))
    io["tw2T"] = din("tw2T", (H, H), BF16)
    io["tb2"] = din("tb2", (H,))
    io["tw3T"] = din("tw3T", (H, S * S), BF16)
    io["tb3"] = din("tb3_bf", (S * S,), BF16)
    io["fw1T"] = din("fw1T", (D, H))
    io["fb1"] = din("fb1_", (H,))
    io["fw2T"] = din("fw2T", (H, H))
    io["fb2"] = din("fb2_", (H,))
    io["mwT"] = din("mwT", (H, D))
    io["mb"] = din("mb_", (D,))
    io["lwT"] = din("lwT", (H, D))
    io["lb"] = din("lb_", (D,))
    io["L1"] = din("L1", (D, S))
    io["L2m"] = din("L2m", (D, S))
    io["L3"] = din("L3", (D, S))
    io["olv_bias"] = din("olv_bias", (S,))

    io["out"] = nc.dram_tensor("ll_out", [B], F32, kind="ExternalOutput").ap()
    dbg = {"kind": "ExternalOutput"} if DEBUG else {}
    io["Ebuf"] = nc.dram_tensor("Ebuf", [B, SLAB, S * S], BF16, **dbg).ap()
    io["Rbuf"] = nc.dram_tensor("Rbuf", [B, S, SLAB], F32, **dbg).ap()   # invR
    io["Hbuf"] = nc.dram_tensor("Hbuf", [B, S, SLAB], F32, **dbg).ap()   # ehat
    io["contrib"] = nc.dram_tensor("contrib", [CONTRIB], F32).ap()
    if DEBUG:
        io["contrib_dbg"] = nc.dram_tensor(
            "contrib_dbg", [CONTRIB], F32, kind="ExternalOutput"
        ).ap()
    io["gathered"] = nc.dram_tensor(
        "gathered", [NCORES * CONTRIB], F32, addr_space="Shared"
    ).ap()

    with tile.TileContext(nc) as tc:
        build_tile(tc, io)
    nc.compile()
    return nc


def build_tile(tc, io):
    nc = tc.nc
    dma = nc.sync
    with (
        tc.tile_pool(name="wts", bufs=1) as wts,
        tc.tile_pool(name="consts", bufs=1) as consts,
    ):
        ident = consts.tile([128, 128], F32)
        make_identity(nc, ident)
        identb = consts.tile([128, 128], BF16)
        nc.vector.tensor_copy(identb, ident)

        def load_w(tag, ap_dram, shape, dtype=F32):
            t = wts.tile(list(shape), dtype, tag=tag)
            dma.dma_start(out=t, in_=ap_dram)
            return t

        w = {}
        w["tw1cT"] = load_w("w1", io["tw1cT"], (C, H), BF16)
        w["tw2T"] = [load_w(f"w2{k}", io["tw2T"][k * 128:(k + 1) * 128, :],
                            (128, H), BF16) for k in range(2)]
        w["tw3T"] = [load_w(f"w3{k}", io["tw3T"][k * 128:(k + 1) * 128, :],
                            (128, S * S), BF16) for k in range(2)]
        w["tb3"] = load_w("b3", io["tb3"][None, :], (1, S * S), BF16)
        w["fw1T"] = load_w("g1", io["fw1T"], (D, H))
        w["fw2T"] = [load_w(f"g2{k}", io["fw2T"][k * 128:(k + 1) * 128, :],
                            (128, H)) for k in range(2)]
        w["mwT"] = [load_w(f"gm{k}", io["mwT"][k * 128:(k + 1) * 128, :],
                           (128, D)) for k in range(2)]
        w["lwT"] = [load_w(f"gl{k}", io["lwT"][k * 128:(k + 1) * 128, :],
                           (128, D)) for k in range(2)]
        w["L1"] = load_w("L1", io["L1"], (D, S))
        w["L2m"] = load_w("L2m", io["L2m"], (D, S))
        w["L3"] = load_w("L3", io["L3"], (D, S))
        for nm in ("tb1p", "tb2", "fb1", "fb2"):
            w[nm] = [load_w(f"{nm}{k}", io[nm][k * 128:(k + 1) * 128, None],
                            (128, 1)) for k in range(2)]
        w["mb"] = load_w("mb", io["mb"][:, None], (D, 1))
        w["lb"] = load_w("lb", io["lb"][:, None], (D, 1))
        w["olvb"] = load_w("olvb", io["olv_bias"][:, None], (S, 1))
        w["alpha"] = load_w("alpha", io["alpha"], (S, NCHAIN))

        ones_bt = consts.tile([1, NTILE], BF16)      # lhsT for tb3 rank-1
        nc.vector.memset(ones_bt, 1.0)
        ones80 = consts.tile([D, 1], F32)
        nc.vector.memset(ones80, 1.0)
        pones_row = consts.tile([1, S], F32)         # +1 lhsT (sum_blv rank-1)
        nc.vector.memset(pones_row, 1.0)
        nones_row = consts.tile([1, S], F32)         # -1 lhsT (mean bcast)
        nc.vector.memset(nones_row, -1.0)
        invS_col = consts.tile([S, 1], F32)
        nc.vector.memset(invS_col, 1.0 / S)
        w.update(ones_bt=ones_bt, ones80=ones80, pones_row=pones_row,
                 nones_row=nones_row, invS_col=invS_col, ident=ident,
                 identb=identb)

        nu_sb = consts.tile([1, B], F32)
        a0_sb = consts.tile([S, B], F32)
        lam_sb = consts.tile([1, NCHAIN], F32)
        msl_sb = consts.tile([1, NCHAIN, NSLOT], F32)
        nc.vector.memset(msl_sb, 1.0)
        nslots_sb = consts.tile([1, B, NT_TILES], F32)

        # ================= PHASE 1 =================
        with (
            tc.tile_pool(name="p1", bufs=3) as p1,
            tc.tile_pool(name="p1es", bufs=3) as p1es,
            tc.tile_pool(name="pbig", bufs=3, space="PSUM") as pbig,
            tc.tile_pool(name="plg", bufs=2, space="PSUM") as plg,
            tc.tile_pool(name="psm", bufs=3, space="PSUM") as psm,
        ):
            for b in range(B):
                for hh in range(NT_TILES):
                    phase1_tile(nc, b, hh, dma, p1, p1es, pbig, plg, psm,
                                w, io, nu_sb, a0_sb, nslots_sb)
            if P1_STAGE >= 4:
                for b in range(B):
                    nc.vector.reduce_sum(nu_sb[:, b:b + 1], nslots_sb[:, b, :],
                                         axis=AX.X)

        if BUILD_STAGE < 2:
            return
        # ================= PHASE 2 =================
        with (
            tc.tile_pool(name="p2e", bufs=2) as p2e,
            tc.tile_pool(name="p2s", bufs=2) as p2s,
            tc.tile_pool(name="p2x", bufs=2) as p2x,
            tc.tile_pool(name="p2m", bufs=1) as p2m,
            tc.tile_pool(name="p2ps", bufs=2, space="PSUM") as p2ps,
            tc.tile_pool(name="p2psr", bufs=2, space="PSUM") as p2psr,
        ):
            mfinal = run_chains(nc, dma, p2e, p2s, p2x, p2m, p2ps, p2psr,
                                w, io, msl_sb)

            lnms = p2m.tile([1, NCHAIN, NSLOT], F32)
            nc.scalar.activation(lnms, msl_sb, AF.Ln)
            for c in range(NCHAIN):
                nc.vector.reduce_sum(lam_sb[:, c:c + 1], lnms[:, c, :], axis=AX.X)

            if BUILD_STAGE < 3:
                return
            contrib = io["contrib"]
            for c in range(NCHAIN):
                dma.dma_start(
                    out=contrib[OFF_M + c * S * S:OFF_M + (c + 1) * S * S]
                    .rearrange("(j c2) -> j c2", j=S),
                    in_=mfinal[:, c, :],
                )
            dma.dma_start(out=contrib[OFF_NU:OFF_NU + B][None, :], in_=nu_sb)
            dma.dma_start(out=contrib[OFF_LAM:OFF_LAM + NCHAIN][None, :],
                          in_=lam_sb)
            dma.dma_start(
                out=contrib[OFF_A0:OFF_A0 + B * S]
                .rearrange("(s b2) -> s b2", s=S),
                in_=a0_sb,
            )
            if DEBUG:
                dma.dma_start(out=io["contrib_dbg"], in_=contrib)
            with tc.tile_critical():
                with nc.semaphore("ccsem") as ccsem:
                    nc.gpsimd.collective_compute(
                        "AllGather",
                        ALU.bypass,
                        replica_groups=[list(range(NCORES))],
                        ins=[contrib],
                        outs=[io["gathered"]],
                    ).then_inc(ccsem, 1)
                    nc.gpsimd.wait_ge(ccsem, 1)

        if BUILD_STAGE < 4:
            return
        # ================= COMBINE =================
        with (
            tc.tile_pool(name="cmb", bufs=2) as cmb,
            tc.tile_pool(name="cmbps", bufs=2, space="PSUM") as cmbps,
        ):
            g2 = io["gathered"].rearrange("(k f) -> k f", k=NCORES)
            out_row = cmb.tile([1, B], F32, tag="outrow")
            for b in range(B):
                u = cmb.tile([S, 1], F32, tag="u")
                nc.vector.memset(u, 1.0)
                cslot = cmb.tile([1, 2 * NCORES], F32, tag="cslot")
                # global chunk r = 2*core + sub; chain index on core = sub*4+b
                for step, r in enumerate(range(2 * NCORES - 1, -1, -1)):
                    core, sub = r // 2, r % 2
                    ci = sub * 4 + b
                    m_sb = cmb.tile([S, S + 1], F32, tag="m")
                    nc.vector.memset(m_sb[:, S:S + 1], 1.0)
                    dma.dma_start(
                        out=m_sb[:, 0:S],
                        in_=g2[core, OFF_M + ci * S * S:OFF_M + (ci + 1) * S * S]
                        .rearrange("(j c2) -> j c2", j=S),
                    )
                    up = cmbps.tile([S + 1, 1], F32, tag="up")
                    nc.tensor.matmul(up, m_sb, u)
                    # normalize by input mass (row S) to stay in fp32 range
                    nc.scalar.copy(cslot[:, step:step + 1], up[S:S + 1, 0:1])
                    minv = cmb.tile([1, 1], F32, tag="minv2")
                    nc.vector.reciprocal(minv, up[S:S + 1, 0:1])
                    rbc = cmbps.tile([S, 1], F32, tag="rbc2")
                    nc.tensor.matmul(rbc, w["pones_row"], minv)
                    rbcs = cmb.tile([S, 1], F32, tag="rbcs")
                    nc.scalar.copy(rbcs, rbc)
                    u = cmb.tile([S, 1], F32, tag="u")
                    nc.vector.tensor_mul(u, up[0:S, :], rbcs)
                lncs = cmb.tile([1, 2 * NCORES], F32, tag="lncs")
                nc.scalar.activation(lncs, cslot, AF.Ln)
                csum = cmb.tile([1, 1], F32, tag="csum")
                nc.vector.reduce_sum(csum, lncs, axis=AX.X)
                a0c = cmb.tile([S, 1], F32, tag="a0c")
                a0v = g2[0, OFF_A0:OFF_A0 + B * S].rearrange(
                    "(s b2) -> s b2", s=S
                )
                dma.dma_start(out=a0c, in_=a0v[:, b:b + 1])
                dotp = cmbps.tile([1, 1], F32, tag="dot")
                nc.tensor.matmul(dotp, u, a0c)

                nurow = cmb.tile([1, NCORES], F32, tag="nur")
                dma.dma_start(out=nurow, in_=g2[:, OFF_NU + b][None, :])
                lamrow = cmb.tile([1, 2 * NCORES], F32, tag="lamr")
                dma.dma_start(
                    out=lamrow[:, 0:NCORES],
                    in_=g2[:, OFF_LAM + b][None, :],
                )
                dma.dma_start(
                    out=lamrow[:, NCORES:],
                    in_=g2[:, OFF_LAM + 4 + b][None, :],
                )
                nusum = cmb.tile([1, 1], F32, tag="nus")
                nc.vector.reduce_sum(nusum, nurow, axis=AX.X)
                lamsum = cmb.tile([1, 1], F32, tag="lams")
                nc.vector.reduce_sum(lamsum, lamrow, axis=AX.X)
                lnv = cmb.tile([1, 1], F32, tag="lnv")
                nc.scalar.activation(lnv, dotp, AF.Ln)
                acc = cmb.tile([1, 1], F32, tag="acc")
                nc.vector.tensor_add(acc, lnv, nusum)
                acc2 = cmb.tile([1, 1], F32, tag="acc2")
                nc.vector.tensor_add(acc2, acc, lamsum)
                acc3 = cmb.tile([1, 1], F32, tag="acc3")
                nc.vector.tensor_add(acc3, acc2, csum)
                nc.vector.tensor_scalar_add(out_row[:, b:b + 1], acc3,
                                            -math.log(S))
            dma.dma_start(out=io["out"][None, :], in_=out_row)


def phase1_tile(nc, b, hh, dma, p1, p1es, pbig, plg, psm, w, io,
                nu_sb, a0_sb, nslots_sb):
    nt = NTILE
    t0 = hh * NTILE
    ident = w["ident"]

    ctx_t = p1.tile([nt, C], BF16, tag="ctxt")
    dma.dma_start(out=ctx_t, in_=io["ctx"][b, t0:t0 + nt, :])
    obs_t = p1.tile([nt, D], F32, tag="obst")
    dma.dma_start(out=obs_t, in_=io["obs"][b, t0:t0 + nt, :])

    ctxT_ps = pbig.tile([C, nt], BF16, tag="pp")
    nc.tensor.transpose(ctxT_ps, ctx_t, w["identb"])
    ctxT = p1.tile([C, nt], BF16, tag="ctxT")
    nc.vector.tensor_copy(ctxT, ctxT_ps)

    obsT_ps = psm.tile([D, nt], F32, tag="sm")
    nc.tensor.transpose(obsT_ps, obs_t, ident)
    obsT = p1.tile([D, nt], F32, tag="obsT")
    nc.vector.tensor_copy(obsT, obsT_ps)

    if P1_STAGE < 2:
        return
    # transition MLP (feature-on-partition, bf16)
    h1 = []
    for m in range(2):
        ps = pbig.tile([128, nt], F32, tag="pp")
        nc.tensor.matmul(ps, w["tw1cT"][:, m * 128:(m + 1) * 128], ctxT)
        sb = p1.tile([128, nt], BF16, tag=f"h1_{m}")
        nc.scalar.activation(sb, ps, AF.Relu, bias=w["tb1p"][m], scale=1.0)
        h1.append(sb)
    h2 = []
    for m in range(2):
        ps = pbig.tile([128, nt], F32, tag="pp")
        for k in range(2):
            nc.tensor.matmul(ps, w["tw2T"][k][:, m * 128:(m + 1) * 128], h1[k],
                             start=(k == 0), stop=(k == 1))
        sb = p1.tile([128, nt], BF16, tag=f"h2_{m}")
        nc.scalar.activation(sb, ps, AF.Relu, bias=w["tb2"][m], scale=1.0)
        h2.append(sb)

    if P1_STAGE < 3:
        return
    # logits slices -> exp (bf16) -> Ebuf; R accumulation
    R_sb = p1.tile([nt, S], F32, tag="Rsb")
    for sl in range((S * S) // LSLICE):
        ps = plg.tile([nt, LSLICE], F32, tag="lgp")
        for k in range(2):
            nc.tensor.matmul(ps, h2[k],
                             w["tw3T"][k][:, sl * LSLICE:(sl + 1) * LSLICE],
                             start=(k == 0), stop=False)
        nc.tensor.matmul(ps, w["ones_bt"],
                         w["tb3"][:, sl * LSLICE:(sl + 1) * LSLICE],
                         start=False, stop=True)
        esl = p1es.tile([nt, LSLICE], BF16, tag="esl")
        nc.scalar.activation(esl, ps, AF.Exp)
        dma.dma_start(
            out=io["Ebuf"][b, t0:t0 + nt, sl * LSLICE:(sl + 1) * LSLICE],
            in_=esl,
        )
        ni = LSLICE // S
        nc.vector.reduce_sum(
            R_sb[:, sl * ni:(sl + 1) * ni],
            esl.rearrange("p (i j) -> p i j", j=S),
            axis=AX.X,
        )

    RT_ps = psm.tile([S, nt], F32, tag="sm")
    nc.tensor.transpose(RT_ps, R_sb, ident)
    invR = p1.tile([S, nt], F32, tag="invR")
    nc.vector.reciprocal(invR, RT_ps)
    dma.dma_start(out=io["Rbuf"][b, :, t0:t0 + nt], in_=invR)

    if P1_STAGE < 4:
        return
    # observation model (fp32)
    f1 = []
    for m in range(2):
        ps = pbig.tile([128, nt], F32, tag="pp")
        nc.tensor.matmul(ps, w["fw1T"][:, m * 128:(m + 1) * 128], obsT)
        sb = p1.tile([128, nt], F32, tag=f"f1_{m}")
        nc.scalar.activation(sb, ps, AF.Relu, bias=w["fb1"][m], scale=1.0)
        f1.append(sb)
    f2 = []
    for m in range(2):
        ps = pbig.tile([128, nt], F32, tag="pp")
        for k in range(2):
            nc.tensor.matmul(ps, w["fw2T"][k][:, m * 128:(m + 1) * 128], f1[k],
                             start=(k == 0), stop=(k == 1))
        sb = p1.tile([128, nt], F32, tag=f"f2_{m}")
        nc.scalar.activation(sb, ps, AF.Relu, bias=w["fb2"][m], scale=1.0)
        f2.append(sb)

    bm_ps = psm.tile([D, nt], F32, tag="sm")
    for k in range(2):
        nc.tensor.matmul(bm_ps, w["mwT"][k], f2[k], start=(k == 0), stop=(k == 1))
    bm = p1.tile([D, nt], F32, tag="bm")
    nc.scalar.activation(bm, bm_ps, AF.Identity, bias=w["mb"], scale=1.0)

    blv_ps = psm.tile([D, nt], F32, tag="sm")
    for k in range(2):
        nc.tensor.matmul(blv_ps, w["lwT"][k], f2[k], start=(k == 0), stop=(k == 1))
    blv = p1.tile([D, nt], F32, tag="blv")
    nc.scalar.activation(blv, blv_ps, AF.Identity, bias=w["lb"], scale=1.0)

    r_ = p1.tile([D, nt], F32, tag="r_")
    nc.vector.tensor_sub(r_, obsT, bm)
    e_ = p1.tile([D, nt], F32, tag="e_")
    nc.scalar.activation(e_, blv, AF.Exp, scale=-1.0)
    rr = p1.tile([D, nt], F32, tag="rr")
    nc.vector.tensor_mul(rr, r_, r_)
    A_ = p1.tile([D, nt], F32, tag="A_")
    nc.vector.tensor_mul(A_, rr, e_)
    Bm_ = p1.tile([D, nt], F32, tag="Bm_")
    nc.vector.tensor_mul(Bm_, r_, e_)

    sb_ps = psm.tile([1, nt], F32, tag="sm")
    nc.tensor.matmul(sb_ps, w["ones80"], blv)
    sblv = p1.tile([1, nt], F32, tag="sblv")
    nc.scalar.copy(sblv, sb_ps)

    q_ps = psm.tile([S, nt], F32, tag="sm")
    nc.tensor.matmul(q_ps, w["L1"], A_, start=True, stop=False)
    nc.tensor.matmul(q_ps, w["L2m"], Bm_, start=False, stop=False)
    nc.tensor.matmul(q_ps, w["L3"], e_, start=False, stop=False)
    nc.tensor.matmul(q_ps, w["pones_row"], sblv, start=False, stop=True)

    lp0 = p1.tile([S, nt], F32, tag="lp0")
    nc.scalar.activation(lp0, q_ps, AF.Identity, bias=w["olvb"], scale=-0.5)

    n_ps = psm.tile([1, nt], F32, tag="sm")
    nc.tensor.matmul(n_ps, w["invS_col"], lp0)
    n_sb = p1.tile([1, nt], F32, tag="nsb")
    nc.scalar.copy(n_sb, n_ps)
    d_ps = psm.tile([S, nt], F32, tag="sm")
    nc.tensor.matmul(d_ps, w["nones_row"], n_sb)      # = -n broadcast
    dd = p1.tile([S, nt], F32, tag="dd")
    nc.vector.tensor_add(dd, lp0, d_ps)
    ehat = p1.tile([S, nt], F32, tag="ehat")
    nc.scalar.activation(ehat, dd, AF.Exp)
    dma.dma_start(out=io["Hbuf"][b, :, t0:t0 + nt], in_=ehat)

    if hh == 0:
        nc.vector.tensor_copy(a0_sb[:, b:b + 1], ehat[:, 0:1])
    nc.vector.reduce_sum(nslots_sb[:, b, hh:hh + 1], n_sb, axis=AX.X)


def run_chains(nc, dma, p2e, p2s, p2x, p2m, p2ps, p2psr, w, io, msl_sb):
    """8 interleaved chains; chain c handles (b = c%4, sub = c//4),
    chunk = local t in [sub*128, sub*128+128)."""
    ident = w["ident"]
    alpha = w["alpha"]
    Ebuf, Rbuf, Hbuf = io["Ebuf"], io["Rbuf"], io["Hbuf"]

    # X state packed (64, NCHAIN, 64) bf16; init X_c = diag(1+a*(invR_lo-1))
    xall = p2x.tile([S, NCHAIN, S], BF16, tag="xall")
    for c in range(NCHAIN):
        b, sub = c % 4, c // 4
        lo = sub * CHUNK
        ir0 = p2s.tile([S, 1], F32, tag="ir0")
        dma.dma_start(out=ir0, in_=Rbuf[b, :, lo:lo + 1])
        t1 = p2s.tile([S, 1], F32, tag="ir0a")
        nc.vector.tensor_scalar_add(t1, ir0, -1.0)
        t2 = p2s.tile([S, 1], F32, tag="ir0b")
        nc.vector.tensor_mul(t2, t1, alpha[:, c:c + 1])
        t3 = p2s.tile([S, 1], F32, tag="ir0c")
        nc.vector.tensor_scalar_add(t3, t2, 1.0)
        nc.vector.tensor_scalar_mul(xall[:, c, :], ident[:S, :S], t3)

    eseg = [None] * NCHAIN
    sround = None
    slot_ctr = [0] * NCHAIN
    ev = [Ebuf[b].rearrange("t (i j) -> i t j", i=S) for b in range(B)]

    for k in range(CHUNK):
        if k % SEG == 0:
            sround = p2s.tile([S, NCHAIN, SEG], F32, tag="sround")
            for c in range(NCHAIN):
                b, sub = c % 4, c // 4
                lt0 = sub * CHUNK + k
                et = p2e.tile([S, SEG, S + 1], BF16, tag=f"eseg{c}")
                nc.vector.memset(et[:, :, S:S + 1], 1.0)
                dma.dma_start(out=et[:, :, 0:S], in_=ev[b][:, lt0:lt0 + SEG, :])
                eseg[c] = et
                hseg = p2s.tile([S, SEG], F32, tag=f"hseg{c}")
                dma.dma_start(out=hseg, in_=Hbuf[b, :, lt0:lt0 + SEG])
                last_seg = (k + SEG == CHUNK)
                ncols = SEG - 1 if last_seg else SEG
                irsh = p2s.tile([S, SEG], F32, tag=f"irsh{c}")
                dma.dma_start(out=irsh[:, 0:ncols],
                              in_=Rbuf[b, :, lt0 + 1:lt0 + 1 + ncols])
                nc.vector.tensor_mul(sround[:, c, 0:ncols], hseg[:, 0:ncols],
                                     irsh[:, 0:ncols])
                if last_seg:
                    nc.vector.tensor_copy(sround[:, c, SEG - 1:SEG],
                                          hseg[:, SEG - 1:SEG])
                if k == 0:
                    # chunk-start blend (no-op when alpha==1):
                    # s0 = (1 + a*(ehat0-1)) * invR_1 ; E0 = I + a*(E0-I)
                    b1 = p2s.tile([S, 1], F32, tag="bl1")
                    nc.vector.tensor_scalar_add(b1, hseg[:, 0:1], -1.0)
                    b2 = p2s.tile([S, 1], F32, tag="bl2")
                    nc.vector.tensor_mul(b2, b1, alpha[:, c:c + 1])
                    b3 = p2s.tile([S, 1], F32, tag="bl3")
                    nc.vector.tensor_scalar_add(b3, b2, 1.0)
                    nc.vector.tensor_mul(sround[:, c, 0:1], b3, irsh[:, 0:1])
                    identb = w["identb"]
                    dE = p2s.tile([S, S], BF16, tag="dE")
                    nc.vector.tensor_sub(dE, et[:, 0, 0:S], identb[:S, :S])
                    dEs = p2s.tile([S, S], BF16, tag="dEs")
                    nc.vector.tensor_scalar_mul(dEs, dE, alpha[:, c:c + 1])
                    nc.vector.tensor_add(et[:, 0, 0:S], dEs, identb[:S, :S])

        tt = k % SEG
        ps = p2ps.tile([S + 1, NCHAIN * S], F32, tag="ps")
        for c in range(NCHAIN):
            nc.tensor.matmul(ps[:, c * S:(c + 1) * S], eseg[c][:, tt, :],
                             xall[:, c, :])
        new_x = p2x.tile([S, NCHAIN, S], BF16, tag="xall")
        for c in range(NCHAIN):
            nc.vector.tensor_scalar_mul(
                new_x[:, c, :], ps[0:S, c * S:(c + 1) * S],
                sround[:, c, tt:tt + 1],
            )
        xall = new_x

        for c in range(NCHAIN):
            if k % RENORM_K == (2 * c) % RENORM_K and 0 < k < CHUNK - 1:
                slot = slot_ctr[c]
                slot_ctr[c] += 1
                nc.scalar.copy(msl_sb[:, c, slot:slot + 1],
                               ps[S:S + 1, c * S:c * S + 1])
                minv = p2s.tile([1, 1], F32, tag="minv")
                nc.vector.reciprocal(minv, ps[S:S + 1, c * S:c * S + 1])
                rbc = p2psr.tile([S, 1], F32, tag="rbc")
                nc.tensor.matmul(rbc, w["pones_row"], minv)
                nc.vector.tensor_mul(sround[:, c, tt + 1:tt + 2],
                                     sround[:, c, tt + 1:tt + 2], rbc)

    mfinal = p2m.tile([S, NCHAIN, S], F32, tag="mfinal")
    nc.vector.tensor_copy(mfinal, xall)
    return mfinal


# ======================================================================
# host side
# ======================================================================
_PROGRAM_CACHE = {}


def _get_program():
    if "nc" not in _PROGRAM_CACHE:
        _PROGRAM_CACHE["nc"] = build_program()
    return _PROGRAM_CACHE["nc"]


def host_prep(inp):
    f32 = np.float32
    bf = ml_dtypes.bfloat16
    p = {}
    tw1 = np.asarray(inp["tw1"], f32)
    p["tw1cT"] = np.ascontiguousarray(tw1[:, :C].T).astype(bf)
    p["tb1p"] = (np.asarray(inp["tb1"], f32) + tw1[:, C:].sum(1) / S).astype(f32)
    p["tw2T"] = np.ascontiguousarray(np.asarray(inp["tw2"], f32).T).astype(bf)
    p["tb2"] = np.asarray(inp["tb2"], f32)
    p["tw3T"] = np.ascontiguousarray(np.asarray(inp["tw3"], f32).T).astype(bf)
    p["tb3_bf"] = np.asarray(inp["tb3"], f32).astype(bf)
    p["fw1T"] = np.ascontiguousarray(np.asarray(inp["fw1"], f32).T)
    p["fb1_"] = np.asarray(inp["fb1"], f32)
    p["fw2T"] = np.ascontiguousarray(np.asarray(inp["fw2"], f32).T)
    p["fb2_"] = np.asarray(inp["fb2"], f32)
    p["mwT"] = np.ascontiguousarray(np.asarray(inp["mw"], f32).T)
    p["mb_"] = np.asarray(inp["mb"], f32)
    p["lwT"] = np.ascontiguousarray(np.asarray(inp["lw"], f32).T)
    p["lb_"] = np.asarray(inp["lb"], f32)
    se = np.asarray(inp["state_emb"], f32)
    off_mean = se @ np.asarray(inp["mw"], f32).T
    off_lv = se @ np.asarray(inp["lw"], f32).T
    E1 = np.exp(-off_lv)
    p["L1"] = np.ascontiguousarray(E1.T)
    p["L2m"] = np.ascontiguousarray((-2.0 * off_mean * E1).T)
    p["L3"] = np.ascontiguousarray((off_mean**2 * E1).T)
    p["olv_bias"] = (
        -0.5 * (D * math.log(2.0 * math.pi) + off_lv.sum(1))
    ).astype(f32)
    return p


def kernel(**inputs):
    nc = _get_program()
    p = host_prep(inputs)
    obs = np.asarray(inputs["observations"], np.float32)
    ctx = np.asarray(inputs["context"], np.float32).astype(ml_dtypes.bfloat16)

    in_maps = []
    for k in range(NCORES):
        t0, t1 = SLAB * k, SLAB * (k + 1)
        alpha = np.ones((S, NCHAIN), np.float32)
        if k == 0:
            alpha[:, 0:4] = 0.0   # sub-0 chains on core 0: step-0 = identity
        m = {
            "obs_slab": np.ascontiguousarray(obs[:, t0:t1, :]),
            "ctx_slab": np.ascontiguousarray(ctx[:, t0:t1, :]),
            "alpha_blend": alpha,
        }
        m.update(p)
        in_maps.append(m)

    res = run_bass_kernel_spmd(nc, in_maps, core_ids=list(range(NCORES)))
    return np.asarray(res.results[0]["ll_out"], np.float32)


if __name__ == "__main__":
    sys.path.insert(0, "/root/problem")
    import reference

    inp = {k: np.asarray(v) for k, v in reference.setup_inputs().items()}
    got = kernel(**inp)
    print("kernel:", got)



# revision 32
# speedup vs baseline: 1.5087x; 1.5087x over previous
"""NeuralHMM forward log-likelihood on 8 Trainium2 NeuronCores.

Strategy (data-parallel over time slabs, associative chunk combine):
  - Core k owns time slab t in [256k, 256(k+1)) for ALL batch elements.
  - Phase 1 (parallel over (b,t)): transition MLP -> E = exp(logits) (bf16)
    written to DRAM in [b][i][t][j] layout (i = from-state) so that the
    phase-2 chain loads are contiguous per partition; per-row sums R ->
    invR and obs-model ehat = exp(obs_lp - n) both kept resident in SBUF
    as [state][b][t]; n (per-t normalizer) accumulated into nu.
  - Phase 2: linear-domain operator chains. Per (b, half-slab) chunk of
    128 steps:  M = prod_t D(s_t) E_t^T,  s_t = ehat_t * invR_{t+1} (last
    step of a chunk uses ehat only; chunk init is D(invR_lo)).  8 chunks
    per core run interleaved; the 8 per-step scale ops are fused into ONE
    DVE instruction via a stride-0 broadcast of the packed scale column.
    Renorm every 32 steps for all chains jointly, with exact log
    accounting.
  - Each core pre-combines its two chunk operators per b (P_b = M_hi M_lo),
    AllGathers the 4 P matrices + scalars, then every core redundantly
    runs the 8-step operator/vector chain:
      ll_b = log(1^T P_7 ... P_0 a0) + sum_t n(b,t) + sum(renorm logs)
             - log(S).
Weight-only reshapes/transposes are precomputed on host in kernel().
"""

import math
import os
import sys

import numpy as np

BUILD_STAGE = int(os.environ.get("NHMM_STAGE", "4"))  # 1=p1, 2=+chains, 3=+cc, 4=all
P1_STAGE = int(os.environ.get("NHMM_P1", "4"))  # 1=loads, 2=+mlp, 3=+logits, 4=all
DEBUG = os.environ.get("NHMM_DEBUG", "0") == "1"

sys.path.insert(0, "/opt/trn_rl_repo")

import ml_dtypes  # noqa: E402

import concourse.bass as bass  # noqa: E402
import concourse.tile as tile  # noqa: E402
from concourse import bacc, mybir  # noqa: E402
from concourse.bass_utils import run_bass_kernel_spmd  # noqa: E402
from concourse.masks import make_identity  # noqa: E402

F32 = mybir.dt.float32
BF16 = mybir.dt.bfloat16
AF = mybir.ActivationFunctionType
AX = mybir.AxisListType
ALU = mybir.AluOpType

B, T, D = 4, 2048, 80
S, H, C = 64, 256, 128
NCORES = 8
SLAB = T // NCORES        # 256 timesteps per core
NTILE = 128               # phase-1 tile width
NT_TILES = SLAB // NTILE  # 2
NCHAIN = 8                # interleaved operator chains per core
CHUNK = SLAB // 2         # 128 steps per chain
SEG = 32                  # steps per E-load segment
NSEG = CHUNK // SEG       # 4 segments per chunk
RENORM_K = 32
NSLOT = CHUNK // RENORM_K  # 4 renorm slots per chain
LSLICE = 512

CONTRIB = B * S * S + B + NCHAIN + B * S
OFF_M = 0
OFF_NU = B * S * S
OFF_LAM = OFF_NU + B
OFF_A0 = OFF_LAM + NCHAIN


def build_program(has_tb3):
    nc = bacc.Bacc(
        "TRN2",
        target_bir_lowering=False,
        debug=False,
        enable_asserts=False,
        num_devices=NCORES,
    )

    def din(name, shape, dtype=F32):
        return nc.dram_tensor(name, list(shape), dtype, kind="ExternalInput").ap()

    io = {}
    io["obs"] = din("obs_slab", (B, SLAB, D))
    io["ctx"] = din("ctx_slab", (B, SLAB, C), BF16)
    io["alpha"] = din("alpha_blend", (S, NCHAIN))
    io["tw1cT"] = din("tw1cT", (C, H), BF16)
    io["tb1p"] = din("tb1p", (H,))
    io["tw2T"] = din("tw2T", (H, H), BF16)
    io["tb2"] = din("tb2", (H,))
    io["tw3T"] = din("tw3T", (H, S * S), BF16)
    io["tb3"] = din("tb3_bf", (S * S,), BF16)
    io["fw1T"] = din("fw1T_bf", (D, H), BF16)
    io["fb1"] = din("fb1_", (H,))
    io["fw2T"] = din("fw2T_bf", (H, H), BF16)
    io["fb2"] = din("fb2_", (H,))
    io["mwT"] = din("mwT_bf", (H, D), BF16)
    io["mb"] = din("mb_", (D,))
    io["lwT"] = din("lwT_bf", (H, D), BF16)
    io["lbn"] = din("lb_neg", (D,))
    io["lwsum"] = din("lwsum", (H,), BF16)
    io["L1"] = din("L1", (D, S))      # -0.5 folded in on host
    io["L2m"] = din("L2m", (D, S))
    io["L3"] = din("L3", (D, S))
    io["olv_bias"] = din("olv_bias", (S,))

    io["out"] = nc.dram_tensor("ll_out", [B], F32, kind="ExternalOutput").ap()
    io["Ebuf"] = nc.dram_tensor("Ebuf", [B, S, SLAB, S], BF16).ap()
    io["contrib"] = nc.dram_tensor("contrib", [CONTRIB], F32).ap()
    if DEBUG:
        io["Ebuf_dbg"] = nc.dram_tensor(
            "Ebuf_dbg", [B, S, SLAB, S], BF16, kind="ExternalOutput").ap()
        io["contrib_dbg"] = nc.dram_tensor(
            "contrib_dbg", [CONTRIB], F32, kind="ExternalOutput").ap()
        io["ehat_dbg"] = nc.dram_tensor(
            "ehat_dbg", [S, B, SLAB], F32, kind="ExternalOutput").ap()
        io["invR_dbg"] = nc.dram_tensor(
            "invR_dbg", [S, B, SLAB], F32, kind="ExternalOutput").ap()
        io["gath_dbg"] = nc.dram_tensor(
            "gath_dbg", [NCORES * CONTRIB], F32, kind="ExternalOutput").ap()
        io["cmb_dbg"] = nc.dram_tensor(
            "cmb_dbg", [B, 8], F32, kind="ExternalOutput").ap()
    io["gathered"] = nc.dram_tensor(
        "gathered", [NCORES * CONTRIB], F32, addr_space="Shared"
    ).ap()

    with tile.TileContext(nc) as tc:
        build_tile(tc, io, has_tb3)
    nc.compile()
    return nc


def build_tile(tc, io, has_tb3):
    nc = tc.nc
    dma = nc.sync
    dma2 = nc.scalar
    with (
        tc.tile_pool(name="wts", bufs=1) as wts,
        tc.tile_pool(name="consts", bufs=1) as consts,
    ):
        ident = consts.tile([128, 128], F32)
        make_identity(nc, ident)
        identb = consts.tile([128, 128], BF16)
        nc.vector.tensor_copy(identb, ident)

        def load_w(tag, ap_dram, shape, dtype=F32, eng=dma):
            t = wts.tile(list(shape), dtype, tag=tag)
            eng.dma_start(out=t, in_=ap_dram)
            return t

        w = {}
        w["tw1cT"] = load_w("w1", io["tw1cT"], (C, H), BF16)
        w["tw2T"] = [load_w(f"w2{k}", io["tw2T"][k * 128:(k + 1) * 128, :],
                            (128, H), BF16) for k in range(2)]
        w["tw3T"] = [load_w(f"w3{k}", io["tw3T"][k * 128:(k + 1) * 128, :],
                            (128, S * S), BF16, eng=dma2) for k in range(2)]
        if has_tb3:
            w["tb3"] = load_w("b3", io["tb3"][None, :], (1, S * S), BF16)
        w["fw1T"] = load_w("g1", io["fw1T"], (D, H), BF16)
        w["fw2T"] = [load_w(f"g2{k}", io["fw2T"][k * 128:(k + 1) * 128, :],
                            (128, H), BF16) for k in range(2)]
        w["mwT"] = [load_w(f"gm{k}", io["mwT"][k * 128:(k + 1) * 128, :],
                           (128, D), BF16) for k in range(2)]
        w["lwT"] = [load_w(f"gl{k}", io["lwT"][k * 128:(k + 1) * 128, :],
                           (128, D), BF16) for k in range(2)]
        w["L1"] = load_w("L1", io["L1"], (D, S))
        w["L2m"] = load_w("L2m", io["L2m"], (D, S))
        w["L3"] = load_w("L3", io["L3"], (D, S))
        for nm in ("tb1p", "tb2", "fb1", "fb2"):
            w[nm] = [load_w(f"{nm}{k}", io[nm][k * 128:(k + 1) * 128, None],
                            (128, 1)) for k in range(2)]
        w["lwsum"] = [load_w(f"lws{k}", io["lwsum"][k * 128:(k + 1) * 128, None],
                             (128, 1), BF16) for k in range(2)]
        w["mb"] = load_w("mb", io["mb"][:, None], (D, 1))
        w["lbn"] = load_w("lbn", io["lbn"][:, None], (D, 1))
        w["olvb"] = load_w("olvb", io["olv_bias"][:, None], (S, 1))
        w["alpha"] = load_w("alpha", io["alpha"], (S, NCHAIN))

        ones_bt = consts.tile([1, NTILE], BF16)      # lhsT for tb3 rank-1
        nc.vector.memset(ones_bt, 1.0)
        pones_row = consts.tile([1, S], F32)         # +1 lhsT (bcast via PE)
        nc.vector.memset(pones_row, 1.0)
        nones_row = consts.tile([1, S], F32)         # -1 lhsT (mean bcast)
        nc.vector.memset(nones_row, -1.0)
        invS_col = consts.tile([S, 1], F32)
        nc.vector.memset(invS_col, 1.0 / S)
        ones_colb = consts.tile([S, 1], BF16)        # mass lhsT (bf16)
        nc.vector.memset(ones_colb, 1.0)
        ones_colf = consts.tile([S, 1], F32)         # mass lhsT (f32)
        nc.vector.memset(ones_colf, 1.0)
        w.update(ones_bt=ones_bt, pones_row=pones_row, nones_row=nones_row,
                 invS_col=invS_col, ones_colb=ones_colb, ones_colf=ones_colf,
                 ident=ident, identb=identb)

        # SBUF-resident per-(state, b, t) tensors shared across phases
        ehat_all = consts.tile([S, B, SLAB], F32)
        invR_all = consts.tile([S, B, SLAB], F32)
        nu_sb = consts.tile([1, B], F32)
        lam_sb = consts.tile([1, NCHAIN], F32)
        msl_sb = consts.tile([1, NCHAIN, NSLOT], F32)
        nc.vector.memset(msl_sb, 1.0)
        nslots_n = consts.tile([1, B, NT_TILES], F32)
        nslots_s = consts.tile([1, B, NT_TILES], F32)

        # ================= PHASE 1 =================
        with (
            tc.tile_pool(name="p1", bufs=2) as p1,
            tc.tile_pool(name="pbig", bufs=3, space="PSUM") as pbig,
            tc.tile_pool(name="plg", bufs=2, space="PSUM") as plg,
            tc.tile_pool(name="psm", bufs=3, space="PSUM") as psm,
        ):
            for hh in range(NT_TILES):
                for b in range(B):
                    phase1_tile(nc, b, hh, dma, dma2, p1, pbig, plg, psm,
                                w, io, ehat_all, invR_all, nslots_n, nslots_s,
                                has_tb3)
            if P1_STAGE >= 4:
                for b in range(B):
                    t1 = p1.tile([1, 1], F32, tag="nut1")
                    nc.vector.reduce_sum(t1, nslots_n[:, b, :], axis=AX.X)
                    t2 = p1.tile([1, 1], F32, tag="nut2")
                    nc.vector.reduce_sum(t2, nslots_s[:, b, :], axis=AX.X)
                    t3 = p1.tile([1, 1], F32, tag="nut3")
                    nc.vector.tensor_scalar_mul(t3, t2, -0.5)
                    nc.vector.tensor_add(nu_sb[:, b:b + 1], t1, t3)
            if DEBUG:
                dma.dma_start(out=io["ehat_dbg"], in_=ehat_all)
                dma.dma_start(out=io["invR_dbg"], in_=invR_all)
                dma.dma_start(out=io["Ebuf_dbg"], in_=io["Ebuf"])

        if BUILD_STAGE < 2:
            return
        # ================= PHASE 2 =================
        with tc.tile_pool(name="p2m", bufs=1) as p2m:
            with (
                tc.tile_pool(name="p2e", bufs=2) as p2e,
                tc.tile_pool(name="p2s", bufs=2) as p2s,
                tc.tile_pool(name="p2x", bufs=2) as p2x,
                tc.tile_pool(name="p2ps", bufs=2, space="PSUM") as p2ps,
                tc.tile_pool(name="p2psr", bufs=2, space="PSUM") as p2psr,
            ):
                mfinal = run_chains(nc, dma, dma2, p2e, p2s, p2x, p2m, p2ps,
                                    p2psr, w, io, msl_sb, ehat_all, invR_all)

                lnms = p2m.tile([1, NCHAIN, NSLOT], F32)
                nc.scalar.activation(lnms, msl_sb, AF.Ln)
                for c in range(NCHAIN):
                    nc.vector.reduce_sum(lam_sb[:, c:c + 1], lnms[:, c, :],
                                         axis=AX.X)

            if BUILD_STAGE < 3:
                return
            # pre-combine the core's two chunks per b: P_b = M_hi @ M_lo
            contrib = io["contrib"]
            with tc.tile_pool(name="pcps", bufs=2, space="PSUM") as pcps:
                pstage = p2m.tile([S, B, S], F32, tag="pstage")
                for b in range(B):
                    xt_ps = pcps.tile([S, S], BF16, tag="xt")
                    nc.tensor.transpose(xt_ps, mfinal[:, 4 + b, :],
                                        identb[:S, :S])
                    xt = p2m.tile([S, S], BF16, tag=f"xt{b}")
                    nc.vector.tensor_copy(xt, xt_ps)
                    p_ps = pcps.tile([S, S], F32, tag="pp")
                    nc.tensor.matmul(p_ps, xt, mfinal[:, b, :])
                    nc.scalar.copy(pstage[:, b, :], p_ps)
                dma.dma_start(
                    out=contrib[OFF_M:OFF_M + B * S * S]
                    .rearrange("(b2 a c) -> a b2 c", b2=B, a=S),
                    in_=pstage,
                )
                dma2.dma_start(out=contrib[OFF_NU:OFF_NU + B][None, :],
                               in_=nu_sb)
                dma2.dma_start(out=contrib[OFF_LAM:OFF_LAM + NCHAIN][None, :],
                               in_=lam_sb)
                dma2.dma_start(
                    out=contrib[OFF_A0:OFF_A0 + B * S]
                    .rearrange("(s b2) -> s b2", s=S),
                    in_=ehat_all[:, :, 0],
                )
            if DEBUG:
                dma.dma_start(out=io["contrib_dbg"], in_=contrib)
            with tc.tile_critical():
                with nc.semaphore("ccsem") as ccsem:
                    nc.gpsimd.collective_compute(
                        "AllGather",
                        ALU.bypass,
                        replica_groups=[list(range(NCORES))],
                        ins=[contrib],
                        outs=[io["gathered"]],
                    ).then_inc(ccsem, 1)
                    nc.gpsimd.wait_ge(ccsem, 1)

        if BUILD_STAGE < 4:
            return
        # ================= COMBINE =================
        with (
            tc.tile_pool(name="cmb", bufs=2) as cmb,
            tc.tile_pool(name="cmbps", bufs=2, space="PSUM") as cmbps,
        ):
            g2 = io["gathered"].rearrange("(k f) -> k f", k=NCORES)
            if DEBUG:
                dma.dma_start(out=io["gath_dbg"], in_=io["gathered"])
            cdbg = None
            if DEBUG:
                cdbg = cmb.tile([1, B, 8], F32, tag="cdbg")
            out_row = cmb.tile([1, B], F32, tag="outrow")
            for b in range(B):
                u = cmb.tile([S, 1], F32, tag=f"u{b}")
                nc.vector.memset(u, 1.0)
                for step, r in enumerate(range(NCORES - 1, -1, -1)):
                    m_sb = cmb.tile([S, S], F32, tag=f"m{b}")
                    dma.dma_start(
                        out=m_sb,
                        in_=g2[r, OFF_M + b * S * S:OFF_M + (b + 1) * S * S]
                        .rearrange("(a c) -> a c", a=S),
                    )
                    up = cmbps.tile([S, 1], F32, tag="up")
                    nc.tensor.matmul(up, m_sb, u)
                    u = cmb.tile([S, 1], F32, tag=f"u{b}")
                    nc.vector.tensor_copy(u, up)
                a0c = cmb.tile([S, 1], F32, tag=f"a0{b}")
                dma2.dma_start(
                    out=a0c,
                    in_=g2[0, OFF_A0:OFF_A0 + B * S]
                    .rearrange("(s b2) -> s b2", s=S)[:, b:b + 1],
                )
                dotp = cmbps.tile([1, 1], F32, tag="dot")
                nc.tensor.matmul(dotp, u, a0c)

                nurow = cmb.tile([1, NCORES], F32, tag=f"nur{b}")
                dma.dma_start(out=nurow, in_=g2[:, OFF_NU + b][None, :])
                lamrow = cmb.tile([1, 2 * NCORES], F32, tag=f"lamr{b}")
                dma.dma_start(
                    out=lamrow[:, 0:NCORES],
                    in_=g2[:, OFF_LAM + b][None, :],
                )
                dma.dma_start(
                    out=lamrow[:, NCORES:],
                    in_=g2[:, OFF_LAM + 4 + b][None, :],
                )
                nusum = cmb.tile([1, 1], F32, tag=f"nus{b}")
                nc.vector.reduce_sum(nusum, nurow, axis=AX.X)
                lamsum = cmb.tile([1, 1], F32, tag=f"lams{b}")
                nc.vector.reduce_sum(lamsum, lamrow, axis=AX.X)
                lnv = cmb.tile([1, 1], F32, tag=f"lnv{b}")
                nc.scalar.activation(lnv, dotp, AF.Ln)
                acc = cmb.tile([1, 1], F32, tag=f"acc{b}")
                nc.vector.tensor_add(acc, lnv, nusum)
                acc2 = cmb.tile([1, 1], F32, tag=f"acc2{b}")
                nc.vector.tensor_add(acc2, acc, lamsum)
                nc.vector.tensor_scalar_add(out_row[:, b:b + 1], acc2,
                                            -math.log(S))
                if DEBUG:
                    nc.scalar.copy(cdbg[:, b, 0:1], lnv)
                    nc.scalar.copy(cdbg[:, b, 1:2], lnv)
                    nc.scalar.copy(cdbg[:, b, 2:3], nusum)
                    nc.scalar.copy(cdbg[:, b, 3:4], lamsum)
                    nc.scalar.copy(cdbg[:, b, 4:5], dotp)
                    nc.scalar.copy(cdbg[:, b, 5:6], dotp)
                    nc.scalar.copy(cdbg[:, b, 6:7], u[0:1, :])
                    nc.scalar.copy(cdbg[:, b, 7:8], a0c[0:1, :])
            if DEBUG:
                dma.dma_start(out=io["cmb_dbg"][None, :, :], in_=cdbg)
            dma.dma_start(out=io["out"][None, :], in_=out_row)


def phase1_tile(nc, b, hh, dma, dma2, p1, pbig, plg, psm, w, io,
                ehat_all, invR_all, nslots_n, nslots_s, has_tb3):
    nt = NTILE
    t0 = hh * NTILE
    ident = w["ident"]

    ctx_t = p1.tile([nt, C], BF16, tag="ctxt")
    dma.dma_start(out=ctx_t, in_=io["ctx"][b, t0:t0 + nt, :])
    obs_t = p1.tile([nt, D], F32, tag="obst")
    dma2.dma_start(out=obs_t, in_=io["obs"][b, t0:t0 + nt, :])

    ctxT_ps = pbig.tile([C, nt], BF16, tag="pp")
    nc.tensor.transpose(ctxT_ps, ctx_t, w["identb"])
    ctxT = p1.tile([C, nt], BF16, tag="ctxT")
    nc.vector.tensor_copy(ctxT, ctxT_ps)

    obsT_ps = psm.tile([D, nt], F32, tag="sm")
    nc.tensor.transpose(obsT_ps, obs_t, ident)
    obsT = p1.tile([D, nt], F32, tag="obsT")
    nc.vector.tensor_copy(obsT, obsT_ps)
    obsT_bf = p1.tile([D, nt], BF16, tag="obsTb")
    nc.scalar.copy(obsT_bf, obsT_ps)

    if P1_STAGE < 2:
        return
    # transition MLP (feature-on-partition, bf16)
    h1 = []
    for m in range(2):
        ps = pbig.tile([128, nt], F32, tag="pp")
        nc.tensor.matmul(ps, w["tw1cT"][:, m * 128:(m + 1) * 128], ctxT)
        sb = p1.tile([128, nt], BF16, tag=f"h1_{m}")
        nc.scalar.activation(sb, ps, AF.Relu, bias=w["tb1p"][m], scale=1.0)
        h1.append(sb)
    h2 = []
    for m in range(2):
        ps = pbig.tile([128, nt], F32, tag="pp")
        for k in range(2):
            nc.tensor.matmul(ps, w["tw2T"][k][:, m * 128:(m + 1) * 128], h1[k],
                             start=(k == 0), stop=(k == 1))
        sb = p1.tile([128, nt], BF16, tag=f"h2_{m}")
        nc.scalar.activation(sb, ps, AF.Relu, bias=w["tb2"][m], scale=1.0)
        h2.append(sb)

    if P1_STAGE < 3:
        return
    # logits slices -> exp (bf16) -> esl_all; R accumulation; E write
    esl_all = p1.tile([nt, S * S], BF16, tag="esl")
    R_sb = p1.tile([nt, S], F32, tag="Rsb")
    nsl = (S * S) // LSLICE
    for sl in range(nsl):
        ps = plg.tile([nt, LSLICE], F32, tag="lgp")
        last = not has_tb3
        for k in range(2):
            nc.tensor.matmul(ps, h2[k],
                             w["tw3T"][k][:, sl * LSLICE:(sl + 1) * LSLICE],
                             start=(k == 0), stop=(k == 1) and last)
        if has_tb3:
            nc.tensor.matmul(ps, w["ones_bt"],
                             w["tb3"][:, sl * LSLICE:(sl + 1) * LSLICE],
                             start=False, stop=True)
        esl = esl_all[:, sl * LSLICE:(sl + 1) * LSLICE]
        nc.scalar.activation(esl, ps, AF.Exp)
        ni = LSLICE // S
        nc.vector.reduce_sum(
            R_sb[:, sl * ni:(sl + 1) * ni],
            esl.rearrange("p (i j) -> p i j", j=S),
            axis=AX.X,
        )
    dma.dma_start(
        out=io["Ebuf"][b, :, t0:t0 + nt, :].rearrange("i t j -> t i j"),
        in_=esl_all.rearrange("t (i j) -> t i j", i=S),
    )

    RT_ps = psm.tile([S, nt], F32, tag="sm")
    nc.tensor.transpose(RT_ps, R_sb, ident)
    nc.vector.reciprocal(invR_all[:, b, t0:t0 + nt], RT_ps)

    if P1_STAGE < 4:
        return
    # observation model (bf16 matmuls, fp32 quadratic form)
    f1 = []
    for m in range(2):
        ps = pbig.tile([128, nt], F32, tag="pp")
        nc.tensor.matmul(ps, w["fw1T"][:, m * 128:(m + 1) * 128], obsT_bf)
        sb = p1.tile([128, nt], BF16, tag=f"f1_{m}")
        nc.vector.tensor_scalar(sb, ps, w["fb1"][m], 0.0, ALU.add, ALU.max)
        f1.append(sb)
    f2 = []
    for m in range(2):
        ps = pbig.tile([128, nt], F32, tag="pp")
        for k in range(2):
            nc.tensor.matmul(ps, w["fw2T"][k][:, m * 128:(m + 1) * 128], f1[k],
                             start=(k == 0), stop=(k == 1))
        sb = p1.tile([128, nt], BF16, tag=f"f2_{m}")
        nc.vector.tensor_scalar(sb, ps, w["fb2"][m], 0.0, ALU.add, ALU.max)
        f2.append(sb)

    bm_ps = psm.tile([D, nt], F32, tag="sm")
    for k in range(2):
        nc.tensor.matmul(bm_ps, w["mwT"][k], f2[k], start=(k == 0), stop=(k == 1))
    blv_ps = psm.tile([D, nt], F32, tag="sm")
    for k in range(2):
        nc.tensor.matmul(blv_ps, w["lwT"][k], f2[k], start=(k == 0), stop=(k == 1))
    sblv_ps = psm.tile([1, nt], F32, tag="sm")
    for k in range(2):
        nc.tensor.matmul(sblv_ps, w["lwsum"][k], f2[k],
                         start=(k == 0), stop=(k == 1))
    nc.vector.reduce_sum(nslots_s[:, b, hh:hh + 1], sblv_ps, axis=AX.X)

    # e_ = exp(-(blv + lb));  r_ = (obs - mb) - bm
    e_ = p1.tile([D, nt], F32, tag="e_")
    nc.scalar.activation(e_, blv_ps, AF.Exp, bias=w["lbn"], scale=-1.0)
    obs2 = p1.tile([D, nt], F32, tag="obs2")
    nc.vector.tensor_scalar_sub(obs2, obsT, w["mb"])
    r_ = p1.tile([D, nt], F32, tag="r_")
    nc.vector.tensor_sub(r_, obs2, bm_ps)
    Bm_ = p1.tile([D, nt], F32, tag="Bm_")
    nc.vector.tensor_mul(Bm_, r_, e_)
    A_ = p1.tile([D, nt], F32, tag="A_")
    nc.vector.tensor_mul(A_, r_, Bm_)

    q_ps = psm.tile([S, nt], F32, tag="sm")
    nc.tensor.matmul(q_ps, w["L1"], A_, start=True, stop=False)
    nc.tensor.matmul(q_ps, w["L2m"], Bm_, start=False, stop=False)
    nc.tensor.matmul(q_ps, w["L3"], e_, start=False, stop=True)

    lp0 = p1.tile([S, nt], F32, tag="lp0")
    nc.scalar.activation(lp0, q_ps, AF.Identity, bias=w["olvb"], scale=1.0)

    n_ps = psm.tile([1, nt], F32, tag="sm")
    nc.tensor.matmul(n_ps, w["invS_col"], lp0)
    n_sb = p1.tile([1, nt], F32, tag="nsb")
    nc.scalar.copy(n_sb, n_ps)
    d_ps = psm.tile([S, nt], F32, tag="sm")
    nc.tensor.matmul(d_ps, w["nones_row"], n_sb)      # = -n broadcast
    dd = p1.tile([S, nt], F32, tag="dd")
    nc.vector.tensor_add(dd, lp0, d_ps)
    nc.scalar.activation(ehat_all[:, b, t0:t0 + nt], dd, AF.Exp)

    nc.vector.reduce_sum(nslots_n[:, b, hh:hh + 1], n_sb, axis=AX.X)
    nc.vector.reduce_sum(nslots_s[:, b, hh:hh + 1], sblv_ps, axis=AX.X)


def run_chains(nc, dma, dma2, p2e, p2s, p2x, p2m, p2ps, p2psr, w, io,
               msl_sb, ehat_all, invR_all):
    """8 interleaved chains; chain c handles (b = c%4, sub = c//4),
    chunk = local t in [sub*128, sub*128+128)."""
    ident = w["ident"]
    identb = w["identb"]
    alpha = w["alpha"]
    Ebuf = io["Ebuf"]

    # X state packed (64, NCHAIN, 64) bf16; init X_c = diag(1+a*(invR_lo-1))
    xall = p2x.tile([S, NCHAIN, S], BF16, tag="xall")
    for c in range(NCHAIN):
        b, sub = c % 4, c // 4
        lo = sub * CHUNK
        t1 = p2s.tile([S, 1], F32, tag="ir0a")
        nc.vector.tensor_scalar_add(t1, invR_all[:, b, lo:lo + 1], -1.0)
        t2 = p2s.tile([S, 1], F32, tag="ir0b")
        nc.vector.tensor_mul(t2, t1, alpha[:, c:c + 1])
        t3 = p2s.tile([S, 1], F32, tag="ir0c")
        nc.vector.tensor_scalar_add(t3, t2, 1.0)
        nc.vector.tensor_scalar_mul(xall[:, c, :], ident[:S, :S], t3)

    eseg = [None] * NCHAIN
    sround = None
    pending_rbc = None

    for k in range(CHUNK):
        if k % SEG == 0:
            sround = p2s.tile([S, NCHAIN, SEG], F32, tag="sround")
            for c in range(NCHAIN):
                b, sub = c % 4, c // 4
                lt0 = sub * CHUNK + k
                et = p2e.tile([S, SEG, S], BF16, tag=f"eseg{c}")
                eng = dma if c % 2 == 0 else dma2
                eng.dma_start(out=et, in_=Ebuf[b, :, lt0:lt0 + SEG, :])
                eseg[c] = et
                last_seg = (k + SEG == CHUNK)
                ncols = SEG - 1 if last_seg else SEG
                nc.vector.tensor_mul(
                    sround[:, c, 0:ncols],
                    ehat_all[:, b, lt0:lt0 + ncols],
                    invR_all[:, b, lt0 + 1:lt0 + 1 + ncols],
                )
                if last_seg:
                    nc.vector.tensor_copy(
                        sround[:, c, SEG - 1:SEG],
                        ehat_all[:, b, lt0 + SEG - 1:lt0 + SEG],
                    )
                if k == 0:
                    # chunk-start blend (no-op when alpha==1):
                    # s0 = (1 + a*(ehat0-1)) * invR_1 ; E0 = I + a*(E0-I)
                    b1 = p2s.tile([S, 1], F32, tag="bl1")
                    nc.vector.tensor_scalar_add(
                        b1, ehat_all[:, b, lt0:lt0 + 1], -1.0)
                    b2 = p2s.tile([S, 1], F32, tag="bl2")
                    nc.vector.tensor_mul(b2, b1, alpha[:, c:c + 1])
                    b3 = p2s.tile([S, 1], F32, tag="bl3")
                    nc.vector.tensor_scalar_add(b3, b2, 1.0)
                    nc.vector.tensor_mul(sround[:, c, 0:1], b3,
                                         invR_all[:, b, lt0 + 1:lt0 + 2])
                    dE = p2s.tile([S, S], BF16, tag="dE")
                    nc.vector.tensor_sub(dE, et[:, 0, :], identb[:S, :S])
                    dEs = p2s.tile([S, S], BF16, tag="dEs")
                    nc.vector.tensor_scalar_mul(dEs, dE, alpha[:, c:c + 1])
                    nc.vector.tensor_add(et[:, 0, :], dEs, identb[:S, :S])
            if pending_rbc is not None:
                nc.vector.tensor_mul(sround[:, :, 0], sround[:, :, 0],
                                     pending_rbc[:, :])
                pending_rbc = None

        tt = k % SEG
        ps = p2ps.tile([S, NCHAIN, S], F32, tag="ps")
        for c in range(NCHAIN):
            nc.tensor.matmul(ps[:, c, :], eseg[c][:, tt, :], xall[:, c, :])
        new_x = p2x.tile([S, NCHAIN, S], BF16, tag="xall")
        i0, i1 = bass.broadcast_tensor_aps(ps[:, :, :],
                                           sround[:, :, tt:tt + 1])
        nc.vector.tensor_tensor(new_x[:, :, :], i0, i1, ALU.mult)
        xall = new_x

        if k % RENORM_K == RENORM_K - 1 and k < CHUNK - 1:
            slot = k // RENORM_K
            mass_ps = p2psr.tile([1, NCHAIN], F32, tag="mass")
            nc.tensor.matmul(mass_ps, w["ones_colb"], xall[:, :, 0:1])
            nc.scalar.copy(msl_sb[:, :, slot], mass_ps)
            minv = p2s.tile([1, NCHAIN], F32, tag="minv")
            nc.vector.reciprocal(minv, mass_ps)
            rbc = p2psr.tile([S, NCHAIN], F32, tag="rbc")
            nc.tensor.matmul(rbc, w["pones_row"], minv)
            if tt + 1 < SEG:
                nc.vector.tensor_mul(sround[:, :, tt + 1],
                                     sround[:, :, tt + 1], rbc[:, :])
            else:
                pending_rbc = rbc

    # final renorm into slot NSLOT-1 so each chunk operator has O(1) mass
    massf = p2psr.tile([1, NCHAIN], F32, tag="mass")
    nc.tensor.matmul(massf, w["ones_colb"], xall[:, :, 0:1])
    nc.scalar.copy(msl_sb[:, :, NSLOT - 1], massf)
    minvf = p2s.tile([1, NCHAIN], F32, tag="minv")
    nc.vector.reciprocal(minvf, massf)
    rbcf = p2psr.tile([S, NCHAIN], F32, tag="rbc")
    nc.tensor.matmul(rbcf, w["pones_row"], minvf)
    mfinal = p2m.tile([S, NCHAIN, S], BF16, tag="mfinal")
    i0, i1 = bass.broadcast_tensor_aps(xall[:, :, :], rbcf[:, :, None])
    nc.vector.tensor_tensor(mfinal[:, :, :], i0, i1, ALU.mult)
    return mfinal


# ======================================================================
# host side
# ======================================================================
_PROGRAM_CACHE = {}


def _get_program(has_tb3):
    key = ("nc", has_tb3)
    if key not in _PROGRAM_CACHE:
        _PROGRAM_CACHE[key] = build_program(has_tb3)
    return _PROGRAM_CACHE[key]


def host_prep(inp):
    f32 = np.float32
    bf = ml_dtypes.bfloat16
    p = {}
    tw1 = np.asarray(inp["tw1"], f32)
    p["tw1cT"] = np.ascontiguousarray(tw1[:, :C].T).astype(bf)
    p["tb1p"] = (np.asarray(inp["tb1"], f32) + tw1[:, C:].sum(1) / S).astype(f32)
    p["tw2T"] = np.ascontiguousarray(np.asarray(inp["tw2"], f32).T).astype(bf)
    p["tb2"] = np.asarray(inp["tb2"], f32)
    p["tw3T"] = np.ascontiguousarray(np.asarray(inp["tw3"], f32).T).astype(bf)
    p["tb3_bf"] = np.asarray(inp["tb3"], f32).astype(bf)
    p["fw1T_bf"] = np.ascontiguousarray(np.asarray(inp["fw1"], f32).T).astype(bf)
    p["fb1_"] = np.asarray(inp["fb1"], f32)
    p["fw2T_bf"] = np.ascontiguousarray(np.asarray(inp["fw2"], f32).T).astype(bf)
    p["fb2_"] = np.asarray(inp["fb2"], f32)
    lw = np.asarray(inp["lw"], f32)
    lb = np.asarray(inp["lb"], f32)
    p["mwT_bf"] = np.ascontiguousarray(np.asarray(inp["mw"], f32).T).astype(bf)
    p["mb_"] = np.asarray(inp["mb"], f32)
    p["lwT_bf"] = np.ascontiguousarray(lw.T).astype(bf)
    p["lb_neg"] = (-lb).astype(f32)
    p["lwsum"] = lw.sum(0).astype(bf)
    se = np.asarray(inp["state_emb"], f32)
    off_mean = se @ np.asarray(inp["mw"], f32).T
    off_lv = se @ lw.T
    E1 = np.exp(-off_lv)
    # -0.5 of the quadratic form folded into the L matrices
    p["L1"] = np.ascontiguousarray((-0.5) * E1.T)
    p["L2m"] = np.ascontiguousarray((-0.5) * (-2.0 * off_mean * E1).T)
    p["L3"] = np.ascontiguousarray((-0.5) * (off_mean**2 * E1).T)
    p["olv_bias"] = (
        -0.5 * (D * math.log(2.0 * math.pi) + off_lv.sum(1) + lb.sum())
    ).astype(f32)
    return p


def build_in_maps(inputs):
    p = host_prep(inputs)
    obs = np.asarray(inputs["observations"], np.float32)
    ctx = np.asarray(inputs["context"], np.float32).astype(ml_dtypes.bfloat16)

    in_maps = []
    for k in range(NCORES):
        t0, t1 = SLAB * k, SLAB * (k + 1)
        alpha = np.ones((S, NCHAIN), np.float32)
        if k == 0:
            alpha[:, 0:4] = 0.0   # sub-0 chains on core 0: step-0 = identity
        m = {
            "obs_slab": np.ascontiguousarray(obs[:, t0:t1, :]),
            "ctx_slab": np.ascontiguousarray(ctx[:, t0:t1, :]),
            "alpha_blend": alpha,
        }
        m.update(p)
        in_maps.append(m)
    return in_maps


def kernel(**inputs):
    has_tb3 = bool(np.any(np.asarray(inputs["tb3"]) != 0))
    nc = _get_program(has_tb3)
    in_maps = build_in_maps(inputs)
    res = run_bass_kernel_spmd(nc, in_maps, core_ids=list(range(NCORES)))
    return np.asarray(res.results[0]["ll_out"], np.float32)


if __name__ == "__main__":
    sys.path.insert(0, "/root/problem")
    import reference

    inp = {k: np.asarray(v) for k, v in reference.setup_inputs().items()}
    got = kernel(**inp)
    print("kernel:", got)


# revision 41
# speedup vs baseline: 1.8962x; 1.2568x over previous
"""NeuralHMM forward log-likelihood on 8 Trainium2 NeuronCores.

Strategy (data-parallel over time slabs, associative chunk combine):
  - Core k owns time slab t in [256k, 256(k+1)) for ALL batch elements.
  - Phase 1 (parallel over (b,t)): transition MLP -> E = exp(logits) (bf16)
    written to DRAM in [b][i][t][j] layout (i = from-state) so that the
    phase-2 chain loads are contiguous per partition; per-row sums R ->
    invR and obs-model ehat = exp(obs_lp - n) both kept resident in SBUF
    as [state][b][t]; n (per-t normalizer) accumulated into nu.
  - Phase 2: linear-domain operator chains. Per (b, half-slab) chunk of
    128 steps:  M = prod_t D(s_t) E_t^T,  s_t = ehat_t * invR_{t+1} (last
    step of a chunk uses ehat only; chunk init is D(invR_lo)).  8 chunks
    per core run interleaved; the 8 per-step scale ops are fused into ONE
    DVE instruction via a stride-0 broadcast of the packed scale column.
    Renorm every 32 steps for all chains jointly, with exact log
    accounting.
  - Each core pre-combines its two chunk operators per b (P_b = M_hi M_lo),
    AllGathers the 4 P matrices + scalars, then every core redundantly
    runs the 8-step operator/vector chain:
      ll_b = log(1^T P_7 ... P_0 a0) + sum_t n(b,t) + sum(renorm logs)
             - log(S).
Weight-only reshapes/transposes are precomputed on host in kernel().
"""

import math
import os
import sys

import numpy as np

BUILD_STAGE = int(os.environ.get("NHMM_STAGE", "4"))  # 1=p1, 2=+chains, 3=+cc, 4=all
P1_STAGE = int(os.environ.get("NHMM_P1", "4"))  # 1=loads, 2=+mlp, 3=+logits, 4=all
DEBUG = os.environ.get("NHMM_DEBUG", "0") == "1"

sys.path.insert(0, "/opt/trn_rl_repo")

import ml_dtypes  # noqa: E402

import concourse.bass as bass  # noqa: E402
import concourse.tile as tile  # noqa: E402
from concourse import bacc, mybir  # noqa: E402
from concourse.bass_utils import run_bass_kernel_spmd  # noqa: E402
from concourse.masks import make_identity  # noqa: E402

F32 = mybir.dt.float32
BF16 = mybir.dt.bfloat16
AF = mybir.ActivationFunctionType
AX = mybir.AxisListType
ALU = mybir.AluOpType

B, T, D = 4, 2048, 80
S, H, C = 64, 256, 128
NCORES = 8
SLAB = T // NCORES        # 256 timesteps per core
NTILE = 128               # phase-1 tile width
NT_TILES = SLAB // NTILE  # 2
NCHAIN = 16               # interleaved operator chains per core
NPAIR = NCHAIN // 2       # chains pair-packed into 128 partitions
CHUNK = SLAB // 4         # 64 steps per chain
SEG = 16                  # steps per E-load segment
NSEG = CHUNK // SEG       # 4 segments per chunk
RENORM_K = 32
NSLOT = 2                 # renorm at k=31 + final normalize
LSLICE = 512

CONTRIB = B * S * S + B + NCHAIN + B * S
OFF_M = 0
OFF_NU = B * S * S
OFF_LAM = OFF_NU + B
OFF_A0 = OFF_LAM + NCHAIN


def build_program(has_tb3):
    nc = bacc.Bacc(
        "TRN2",
        target_bir_lowering=False,
        debug=False,
        enable_asserts=False,
        num_devices=NCORES,
    )

    def din(name, shape, dtype=F32):
        return nc.dram_tensor(name, list(shape), dtype, kind="ExternalInput").ap()

    io = {}
    io["obs"] = din("obs_slab", (B, SLAB, D))
    io["ctx"] = din("ctx_slab", (B, SLAB, C), BF16)
    io["alpha"] = din("alpha_blend", (S, NCHAIN))
    io["alpha2"] = din("alpha_blend2", (2 * S, NPAIR))
    io["mask2T"] = din("mask2T_", (2, 2 * S))
    io["tw1cT"] = din("tw1cT", (C, H), BF16)
    io["tb1p"] = din("tb1p", (H,))
    io["tw2T"] = din("tw2T", (H, H), BF16)
    io["tb2"] = din("tb2", (H,))
    io["tw3T"] = din("tw3T", (H, S * S), BF16)
    io["tb3"] = din("tb3_bf", (S * S,), BF16)
    io["fw1T"] = din("fw1T_bf", (D, H), BF16)
    io["fb1"] = din("fb1_", (H,))
    io["fw2T"] = din("fw2T_bf", (H, H), BF16)
    io["fb2"] = din("fb2_", (H,))
    io["mwT"] = din("mwT_bf", (H, D), BF16)
    io["mb"] = din("mb_", (D,))
    io["lwT"] = din("lwT_bf", (H, D), BF16)
    io["lbn"] = din("lb_neg", (D,))
    io["lwsum"] = din("lwsum", (H,), BF16)
    io["L1"] = din("L1", (D, S))      # -0.5 folded in on host
    io["L2m"] = din("L2m", (D, S))
    io["L3"] = din("L3", (D, S))
    io["olv_bias"] = din("olv_bias", (S,))

    io["out"] = nc.dram_tensor("ll_out", [B], F32, kind="ExternalOutput").ap()
    io["Ebuf"] = nc.dram_tensor("Ebuf", [B, S, SLAB, S], BF16).ap()
    io["contrib"] = nc.dram_tensor("contrib", [CONTRIB], F32).ap()
    if DEBUG:
        io["Ebuf_dbg"] = nc.dram_tensor(
            "Ebuf_dbg", [B, S, SLAB, S], BF16, kind="ExternalOutput").ap()
        io["contrib_dbg"] = nc.dram_tensor(
            "contrib_dbg", [CONTRIB], F32, kind="ExternalOutput").ap()
        io["ehat_dbg"] = nc.dram_tensor(
            "ehat_dbg", [S, B, SLAB], F32, kind="ExternalOutput").ap()
        io["invR_dbg"] = nc.dram_tensor(
            "invR_dbg", [S, B, SLAB], F32, kind="ExternalOutput").ap()
        io["gath_dbg"] = nc.dram_tensor(
            "gath_dbg", [NCORES * CONTRIB], F32, kind="ExternalOutput").ap()
        io["cmb_dbg"] = nc.dram_tensor(
            "cmb_dbg", [B, 8], F32, kind="ExternalOutput").ap()
    io["gathered"] = nc.dram_tensor(
        "gathered", [NCORES * CONTRIB], F32, addr_space="Shared"
    ).ap()

    with tile.TileContext(nc) as tc:
        build_tile(tc, io, has_tb3)
    nc.compile()
    return nc


def build_tile(tc, io, has_tb3):
    nc = tc.nc
    dma = nc.sync
    dma2 = nc.scalar
    with (
        tc.tile_pool(name="wts", bufs=1) as wts,
        tc.tile_pool(name="consts", bufs=1) as consts,
        tc.tile_pool(name="p2w", bufs=1) as p2w,
    ):
        ident = consts.tile([128, 128], F32)
        make_identity(nc, ident)
        identb = consts.tile([128, 128], BF16)
        nc.vector.tensor_copy(identb, ident)

        def load_w(tag, ap_dram, shape, dtype=F32, eng=dma):
            t = wts.tile(list(shape), dtype, tag=tag)
            eng.dma_start(out=t, in_=ap_dram)
            return t

        w = {}
        w["tw1cT"] = load_w("w1", io["tw1cT"], (C, H), BF16)
        w["tw2T"] = [load_w(f"w2{k}", io["tw2T"][k * 128:(k + 1) * 128, :],
                            (128, H), BF16) for k in range(2)]
        w["tw3T"] = [load_w(f"w3{k}", io["tw3T"][k * 128:(k + 1) * 128, :],
                            (128, S * S), BF16, eng=dma2) for k in range(2)]
        if has_tb3:
            w["tb3"] = load_w("b3", io["tb3"][None, :], (1, S * S), BF16)
        w["fw1T"] = load_w("g1", io["fw1T"], (D, H), BF16)
        w["fw2T"] = [load_w(f"g2{k}", io["fw2T"][k * 128:(k + 1) * 128, :],
                            (128, H), BF16) for k in range(2)]
        w["mwT"] = [load_w(f"gm{k}", io["mwT"][k * 128:(k + 1) * 128, :],
                           (128, D), BF16) for k in range(2)]
        w["lwT"] = [load_w(f"gl{k}", io["lwT"][k * 128:(k + 1) * 128, :],
                           (128, D), BF16) for k in range(2)]
        w["L1"] = load_w("L1", io["L1"], (D, S))
        w["L2m"] = load_w("L2m", io["L2m"], (D, S))
        w["L3"] = load_w("L3", io["L3"], (D, S))
        for nm in ("tb1p", "tb2", "fb1", "fb2"):
            w[nm] = [load_w(f"{nm}{k}", io[nm][k * 128:(k + 1) * 128, None],
                            (128, 1)) for k in range(2)]
        w["lwsum"] = [load_w(f"lws{k}", io["lwsum"][k * 128:(k + 1) * 128, None],
                             (128, 1), BF16) for k in range(2)]
        w["mb"] = load_w("mb", io["mb"][:, None], (D, 1))
        w["lbn"] = load_w("lbn", io["lbn"][:, None], (D, 1))
        w["olvb"] = load_w("olvb", io["olv_bias"][:, None], (S, 1))
        w["alpha"] = load_w("alpha", io["alpha"], (S, NCHAIN))
        w["alpha2"] = load_w("alpha2", io["alpha2"], (2 * S, NPAIR))

        ones_bt = consts.tile([1, NTILE], BF16)      # lhsT for tb3 rank-1
        nc.vector.memset(ones_bt, 1.0)
        pones_row = consts.tile([1, S], F32)         # +1 lhsT (bcast via PE)
        nc.vector.memset(pones_row, 1.0)
        nones_row = consts.tile([1, S], F32)         # -1 lhsT (mean bcast)
        nc.vector.memset(nones_row, -1.0)
        invS_col = consts.tile([S, 1], F32)
        nc.vector.memset(invS_col, 1.0 / S)
        ones_colb = consts.tile([S, 1], BF16)        # mass lhsT (bf16)
        nc.vector.memset(ones_colb, 1.0)
        ones_colf = consts.tile([S, 1], F32)         # mass lhsT (f32)
        nc.vector.memset(ones_colf, 1.0)
        mask2 = consts.tile([2 * S, 2], BF16)        # per-half mass lhsT
        nc.vector.memset(mask2, 0.0)
        nc.vector.memset(mask2[0:S, 0:1], 1.0)
        nc.vector.memset(mask2[S:2 * S, 1:2], 1.0)
        mask2T = load_w("mask2T", io["mask2T"], (2, 2 * S))
        w.update(ones_bt=ones_bt, pones_row=pones_row, nones_row=nones_row,
                 invS_col=invS_col, ones_colb=ones_colb, ones_colf=ones_colf,
                 mask2=mask2, mask2T=mask2T, ident=ident, identb=identb)

        # SBUF-resident per-(state, b, t) tensors shared across phases
        ehat_all = consts.tile([S, B, SLAB], F32)
        invR_all = consts.tile([S, B, SLAB], F32)
        nu_sb = consts.tile([1, B], F32)
        lam2 = consts.tile([2, NPAIR], F32)
        msl2 = consts.tile([2, NPAIR, NSLOT], F32)
        nc.vector.memset(msl2, 1.0)
        # persistent pair-packed E tiles (block-diagonal lhsT), double-buffered
        etp = [[None, None] for _ in range(NPAIR)]
        for p in range(NPAIR):
            for par in range(2):
                t = p2w.tile([2 * S, SEG, 2 * S], BF16, tag=f"etp{p}_{par}")
                nc.gpsimd.memset(t, 0.0)
                etp[p][par] = t
        nslots_n = consts.tile([1, B, NT_TILES], F32)
        nslots_s = consts.tile([1, B, NT_TILES], F32)

        # ================= PHASE 1 =================
        with (
            tc.tile_pool(name="p1", bufs=2) as p1,
            tc.tile_pool(name="pbig", bufs=3, space="PSUM") as pbig,
            tc.tile_pool(name="plg", bufs=2, space="PSUM") as plg,
            tc.tile_pool(name="psm", bufs=3, space="PSUM") as psm,
        ):
            for hh in range(NT_TILES):
                for b in range(B):
                    phase1_tile(nc, b, hh, dma, dma2, p1, pbig, plg, psm,
                                w, io, ehat_all, invR_all, nslots_n, nslots_s,
                                has_tb3)
            if P1_STAGE >= 4:
                for b in range(B):
                    t1 = p1.tile([1, 1], F32, tag="nut1")
                    nc.vector.reduce_sum(t1, nslots_n[:, b, :], axis=AX.X)
                    t2 = p1.tile([1, 1], F32, tag="nut2")
                    nc.vector.reduce_sum(t2, nslots_s[:, b, :], axis=AX.X)
                    t3 = p1.tile([1, 1], F32, tag="nut3")
                    nc.vector.tensor_scalar_mul(t3, t2, -0.5)
                    nc.vector.tensor_add(nu_sb[:, b:b + 1], t1, t3)
            if DEBUG:
                dma.dma_start(out=io["ehat_dbg"], in_=ehat_all)
                dma.dma_start(out=io["invR_dbg"], in_=invR_all)
                dma.dma_start(out=io["Ebuf_dbg"], in_=io["Ebuf"])

        if BUILD_STAGE < 2:
            return
        # ================= PHASE 2 =================
        with tc.tile_pool(name="p2m", bufs=1) as p2m:
            with (
                tc.tile_pool(name="p2s", bufs=2) as p2s,
                tc.tile_pool(name="p2x", bufs=2) as p2x,
                tc.tile_pool(name="p2ps", bufs=2, space="PSUM") as p2ps,
                tc.tile_pool(name="p2psr", bufs=1, space="PSUM") as p2psr,
            ):
                mfinal = run_chains(nc, dma, dma2, p2s, p2x, p2m, p2ps,
                                    p2psr, w, io, msl2, ehat_all, invR_all,
                                    etp)

                lnms = p2m.tile([2, NPAIR, NSLOT], F32)
                nc.scalar.activation(lnms, msl2, AF.Ln)
                for p in range(NPAIR):
                    nc.vector.reduce_sum(lam2[:, p:p + 1], lnms[:, p, :],
                                         axis=AX.X)

            if BUILD_STAGE < 3:
                return
            # pre-combine the core's four chunks per b into V_b = P_b^T,
            # P_b = M3 M2 M1 M0  =>  V_b = M0^T M1^T M2^T M3^T
            contrib = io["contrib"]
            with tc.tile_pool(name="pcps", bufs=2, space="PSUM") as pcps:
                pstage = p2m.tile([S, B, S], F32, tag="pstage")
                for b in range(B):
                    # chunk operators: sub0 = (lo, b), sub1 = (lo, 4+b),
                    # sub2 = (up, b), sub3 = (up, 4+b) in mfinal [128, 8, 64]
                    m0 = mfinal[0:S, b, :]
                    m1 = mfinal[0:S, 4 + b, :]
                    m2 = mfinal[S:2 * S, b, :]
                    m3 = mfinal[S:2 * S, 4 + b, :]
                    m3t_ps = pcps.tile([2 * S, S], BF16, tag="xt")
                    nc.tensor.transpose(m3t_ps[S:2 * S, :], m3,
                                        identb[S:2 * S, S:2 * S])
                    m3t = p2m.tile([2 * S, S], BF16, tag=f"m3t{b}")
                    nc.vector.tensor_copy(m3t[S:2 * S, :], m3t_ps[S:2 * S, :])
                    z1_ps = pcps.tile([S, S], F32, tag="pp")
                    nc.tensor.matmul(z1_ps, m2, m3t[S:2 * S, :])
                    z1 = p2m.tile([S, S], BF16, tag=f"z1{b}")
                    nc.vector.tensor_copy(z1, z1_ps)
                    z2_ps = pcps.tile([S, S], F32, tag="pp")
                    nc.tensor.matmul(z2_ps, m1, z1)
                    z2 = p2m.tile([S, S], BF16, tag=f"z2{b}")
                    nc.vector.tensor_copy(z2, z2_ps)
                    v_ps = pcps.tile([S, S], F32, tag="pp")
                    nc.tensor.matmul(v_ps, m0, z2)
                    nc.scalar.copy(pstage[:, b, :], v_ps)
                dma.dma_start(
                    out=contrib[OFF_M:OFF_M + B * S * S]
                    .rearrange("(b2 a c) -> a b2 c", b2=B, a=S),
                    in_=pstage,
                )
                dma2.dma_start(out=contrib[OFF_NU:OFF_NU + B][None, :],
                               in_=nu_sb)
                dma2.dma_start(
                    out=contrib[OFF_LAM:OFF_LAM + NCHAIN]
                    .rearrange("(h p) -> h p", h=2),
                    in_=lam2,
                )
                dma2.dma_start(
                    out=contrib[OFF_A0:OFF_A0 + B * S]
                    .rearrange("(s b2) -> s b2", s=S),
                    in_=ehat_all[:, :, 0],
                )
            if DEBUG:
                dma.dma_start(out=io["contrib_dbg"], in_=contrib)
            with tc.tile_critical():
                with nc.semaphore("ccsem") as ccsem:
                    nc.gpsimd.collective_compute(
                        "AllGather",
                        ALU.bypass,
                        replica_groups=[list(range(NCORES))],
                        ins=[contrib],
                        outs=[io["gathered"]],
                    ).then_inc(ccsem, 1)
                    nc.gpsimd.wait_ge(ccsem, 1)

        if BUILD_STAGE < 4:
            return
        # ================= COMBINE =================
        with (
            tc.tile_pool(name="cmb", bufs=2) as cmb,
            tc.tile_pool(name="cmbps", bufs=2, space="PSUM") as cmbps,
        ):
            g2 = io["gathered"].rearrange("(k f) -> k f", k=NCORES)
            if DEBUG:
                dma.dma_start(out=io["gath_dbg"], in_=io["gathered"])
            cdbg = None
            if DEBUG:
                cdbg = cmb.tile([1, B, 8], F32, tag="cdbg")
            out_row = cmb.tile([1, B], F32, tag="outrow")
            for b in range(B):
                w_ = cmb.tile([S, 1], F32, tag=f"u{b}")
                dma2.dma_start(
                    out=w_,
                    in_=g2[0, OFF_A0:OFF_A0 + B * S]
                    .rearrange("(s b2) -> s b2", s=S)[:, b:b + 1],
                )
                for r in range(NCORES):
                    eng = dma if r % 2 == 0 else dma2
                    m_sb = cmb.tile([S, S], F32, tag=f"m{b}")
                    eng.dma_start(
                        out=m_sb,
                        in_=g2[r, OFF_M + b * S * S:OFF_M + (b + 1) * S * S]
                        .rearrange("(a c) -> a c", a=S),
                    )
                    up = cmbps.tile([S, 1], F32, tag="up")
                    nc.tensor.matmul(up, m_sb, w_)
                    w_ = cmb.tile([S, 1], F32, tag=f"u{b}")
                    nc.vector.tensor_copy(w_, up)
                dotp = cmbps.tile([1, 1], F32, tag="dot")
                nc.tensor.matmul(dotp, w["ones_colf"], w_)

                nurow = cmb.tile([1, NCORES], F32, tag=f"nur{b}")
                dma.dma_start(out=nurow, in_=g2[:, OFF_NU + b][None, :])
                lamrow = cmb.tile([1, 4 * NCORES], F32, tag=f"lamr{b}")
                for q in range(4):
                    dma2.dma_start(
                        out=lamrow[:, q * NCORES:(q + 1) * NCORES],
                        in_=g2[:, OFF_LAM + 4 * q + b][None, :],
                    )
                nusum = cmb.tile([1, 1], F32, tag=f"nus{b}")
                nc.vector.reduce_sum(nusum, nurow, axis=AX.X)
                lamsum = cmb.tile([1, 1], F32, tag=f"lams{b}")
                nc.vector.reduce_sum(lamsum, lamrow, axis=AX.X)
                lnv = cmb.tile([1, 1], F32, tag=f"lnv{b}")
                nc.scalar.activation(lnv, dotp, AF.Ln)
                acc = cmb.tile([1, 1], F32, tag=f"acc{b}")
                nc.vector.tensor_add(acc, lnv, nusum)
                acc2 = cmb.tile([1, 1], F32, tag=f"acc2{b}")
                nc.vector.tensor_add(acc2, acc, lamsum)
                nc.vector.tensor_scalar_add(out_row[:, b:b + 1], acc2,
                                            -math.log(S))
                if DEBUG:
                    nc.scalar.copy(cdbg[:, b, 0:1], lnv)
                    nc.scalar.copy(cdbg[:, b, 1:2], lnv)
                    nc.scalar.copy(cdbg[:, b, 2:3], nusum)
                    nc.scalar.copy(cdbg[:, b, 3:4], lamsum)
                    nc.scalar.copy(cdbg[:, b, 4:5], dotp)
                    nc.scalar.copy(cdbg[:, b, 5:6], dotp)
                    nc.scalar.copy(cdbg[:, b, 6:7], w_[0:1, :])
                    nc.scalar.copy(cdbg[:, b, 7:8], w_[0:1, :])
            if DEBUG:
                dma.dma_start(out=io["cmb_dbg"][None, :, :], in_=cdbg)
            dma.dma_start(out=io["out"][None, :], in_=out_row)


def phase1_tile(nc, b, hh, dma, dma2, p1, pbig, plg, psm, w, io,
                ehat_all, invR_all, nslots_n, nslots_s, has_tb3):
    nt = NTILE
    t0 = hh * NTILE
    ident = w["ident"]

    ctx_t = p1.tile([nt, C], BF16, tag="ctxt")
    dma.dma_start(out=ctx_t, in_=io["ctx"][b, t0:t0 + nt, :])
    obs_t = p1.tile([nt, D], F32, tag="obst")
    dma2.dma_start(out=obs_t, in_=io["obs"][b, t0:t0 + nt, :])

    ctxT_ps = pbig.tile([C, nt], BF16, tag="pp")
    nc.tensor.transpose(ctxT_ps, ctx_t, w["identb"])
    ctxT = p1.tile([C, nt], BF16, tag="ctxT")
    nc.vector.tensor_copy(ctxT, ctxT_ps)

    obsT_ps = psm.tile([D, nt], F32, tag="sm")
    nc.tensor.transpose(obsT_ps, obs_t, ident)
    obsT = p1.tile([D, nt], F32, tag="obsT")
    nc.vector.tensor_copy(obsT, obsT_ps)
    obsT_bf = p1.tile([D, nt], BF16, tag="obsTb")
    nc.scalar.copy(obsT_bf, obsT_ps)

    if P1_STAGE < 2:
        return
    # transition MLP (feature-on-partition, bf16)
    h1 = []
    for m in range(2):
        ps = pbig.tile([128, nt], F32, tag="pp")
        nc.tensor.matmul(ps, w["tw1cT"][:, m * 128:(m + 1) * 128], ctxT)
        sb = p1.tile([128, nt], BF16, tag=f"h1_{m}")
        nc.scalar.activation(sb, ps, AF.Relu, bias=w["tb1p"][m], scale=1.0)
        h1.append(sb)
    h2 = []
    for m in range(2):
        ps = pbig.tile([128, nt], F32, tag="pp")
        for k in range(2):
            nc.tensor.matmul(ps, w["tw2T"][k][:, m * 128:(m + 1) * 128], h1[k],
                             start=(k == 0), stop=(k == 1))
        sb = p1.tile([128, nt], BF16, tag=f"h2_{m}")
        nc.scalar.activation(sb, ps, AF.Relu, bias=w["tb2"][m], scale=1.0)
        h2.append(sb)

    if P1_STAGE < 3:
        return
    # logits slices -> exp (bf16) -> esl_all; R accumulation; E write
    esl_all = p1.tile([nt, S * S], BF16, tag="esl")
    R_sb = p1.tile([nt, S], F32, tag="Rsb")
    nsl = (S * S) // LSLICE
    for sl in range(nsl):
        ps = plg.tile([nt, LSLICE], F32, tag="lgp")
        last = not has_tb3
        for k in range(2):
            nc.tensor.matmul(ps, h2[k],
                             w["tw3T"][k][:, sl * LSLICE:(sl + 1) * LSLICE],
                             start=(k == 0), stop=(k == 1) and last)
        if has_tb3:
            nc.tensor.matmul(ps, w["ones_bt"],
                             w["tb3"][:, sl * LSLICE:(sl + 1) * LSLICE],
                             start=False, stop=True)
        esl = esl_all[:, sl * LSLICE:(sl + 1) * LSLICE]
        nc.scalar.activation(esl, ps, AF.Exp)
        ni = LSLICE // S
        nc.vector.reduce_sum(
            R_sb[:, sl * ni:(sl + 1) * ni],
            esl.rearrange("p (i j) -> p i j", j=S),
            axis=AX.X,
        )
    dma.dma_start(
        out=io["Ebuf"][b, :, t0:t0 + nt, :].rearrange("i t j -> t i j"),
        in_=esl_all.rearrange("t (i j) -> t i j", i=S),
    )

    RT_ps = psm.tile([S, nt], F32, tag="sm")
    nc.tensor.transpose(RT_ps, R_sb, ident)
    nc.vector.reciprocal(invR_all[:, b, t0:t0 + nt], RT_ps)

    if P1_STAGE < 4:
        return
    # observation model (bf16 matmuls, fp32 quadratic form)
    f1 = []
    for m in range(2):
        ps = pbig.tile([128, nt], F32, tag="pp")
        nc.tensor.matmul(ps, w["fw1T"][:, m * 128:(m + 1) * 128], obsT_bf)
        sb = p1.tile([128, nt], BF16, tag=f"f1_{m}")
        nc.vector.tensor_scalar(sb, ps, w["fb1"][m], 0.0, ALU.add, ALU.max)
        f1.append(sb)
    f2 = []
    for m in range(2):
        ps = pbig.tile([128, nt], F32, tag="pp")
        for k in range(2):
            nc.tensor.matmul(ps, w["fw2T"][k][:, m * 128:(m + 1) * 128], f1[k],
                             start=(k == 0), stop=(k == 1))
        sb = p1.tile([128, nt], BF16, tag=f"f2_{m}")
        nc.vector.tensor_scalar(sb, ps, w["fb2"][m], 0.0, ALU.add, ALU.max)
        f2.append(sb)

    bm_ps = psm.tile([D, nt], F32, tag="sm")
    for k in range(2):
        nc.tensor.matmul(bm_ps, w["mwT"][k], f2[k], start=(k == 0), stop=(k == 1))
    blv_ps = psm.tile([D, nt], F32, tag="sm")
    for k in range(2):
        nc.tensor.matmul(blv_ps, w["lwT"][k], f2[k], start=(k == 0), stop=(k == 1))
    sblv_ps = psm.tile([1, nt], F32, tag="sm")
    for k in range(2):
        nc.tensor.matmul(sblv_ps, w["lwsum"][k], f2[k],
                         start=(k == 0), stop=(k == 1))
    nc.vector.reduce_sum(nslots_s[:, b, hh:hh + 1], sblv_ps, axis=AX.X)

    # e_ = exp(-(blv + lb));  r_ = (obs - mb) - bm
    e_ = p1.tile([D, nt], F32, tag="e_")
    nc.scalar.activation(e_, blv_ps, AF.Exp, bias=w["lbn"], scale=-1.0)
    obs2 = p1.tile([D, nt], F32, tag="obs2")
    nc.vector.tensor_scalar_sub(obs2, obsT, w["mb"])
    r_ = p1.tile([D, nt], F32, tag="r_")
    nc.vector.tensor_sub(r_, obs2, bm_ps)
    Bm_ = p1.tile([D, nt], F32, tag="Bm_")
    nc.vector.tensor_mul(Bm_, r_, e_)
    A_ = p1.tile([D, nt], F32, tag="A_")
    nc.vector.tensor_mul(A_, r_, Bm_)

    q_ps = psm.tile([S, nt], F32, tag="sm")
    nc.tensor.matmul(q_ps, w["L1"], A_, start=True, stop=False)
    nc.tensor.matmul(q_ps, w["L2m"], Bm_, start=False, stop=False)
    nc.tensor.matmul(q_ps, w["L3"], e_, start=False, stop=True)

    lp0 = p1.tile([S, nt], F32, tag="lp0")
    nc.scalar.activation(lp0, q_ps, AF.Identity, bias=w["olvb"], scale=1.0)

    n_ps = psm.tile([1, nt], F32, tag="sm")
    nc.tensor.matmul(n_ps, w["invS_col"], lp0)
    n_sb = p1.tile([1, nt], F32, tag="nsb")
    nc.scalar.copy(n_sb, n_ps)
    d_ps = psm.tile([S, nt], F32, tag="sm")
    nc.tensor.matmul(d_ps, w["nones_row"], n_sb)      # = -n broadcast
    dd = p1.tile([S, nt], F32, tag="dd")
    nc.vector.tensor_add(dd, lp0, d_ps)
    nc.scalar.activation(ehat_all[:, b, t0:t0 + nt], dd, AF.Exp)

    nc.vector.reduce_sum(nslots_n[:, b, hh:hh + 1], n_sb, axis=AX.X)
    nc.vector.reduce_sum(nslots_s[:, b, hh:hh + 1], sblv_ps, axis=AX.X)


def run_chains(nc, dma, dma2, p2s, p2x, p2m, p2ps, p2psr, w, io,
               msl2, ehat_all, invR_all, etp):
    """16 chains, pair-packed 2-per-128-partitions; chain c handles
    (b = c%4, sub = c//4), chunk = local t in [sub*64, sub*64+64).
    Pair p stacks chain p (partitions 0:64) and chain 8+p (64:128) with
    block-diagonal E tiles; groups g in {0,1} cover pairs 4g..4g+3."""
    ident = w["ident"]
    identb = w["identb"]
    alpha = w["alpha"]
    alpha2 = w["alpha2"]
    Ebuf = io["Ebuf"]

    def chain_of(p, h):
        return h * NPAIR + p

    def bsub(c):
        return c % 4, c // 4

    # per-chain diag init on lower partitions, then DMA into pair layout
    xinit = p2s.tile([S, NCHAIN, S], BF16, tag="xinit")
    for c in range(NCHAIN):
        b, sub = bsub(c)
        lo = sub * CHUNK
        t1 = p2s.tile([S, 1], F32, tag="ir0a")
        nc.vector.tensor_scalar_add(t1, invR_all[:, b, lo:lo + 1], -1.0)
        t2 = p2s.tile([S, 1], F32, tag="ir0b")
        nc.vector.tensor_mul(t2, t1, alpha[:, c:c + 1])
        t3 = p2s.tile([S, 1], F32, tag="ir0c")
        nc.vector.tensor_scalar_add(t3, t2, 1.0)
        nc.vector.tensor_scalar_mul(xinit[:, c, :], ident[:S, :S], t3)
    xg = [None, None]
    for g in range(2):
        xt = p2x.tile([2 * S, 4, S], BF16, tag=f"xa{g}")
        dma.dma_start(out=xt[0:S, :, :], in_=xinit[:, 4 * g:4 * g + 4, :])
        dma.dma_start(out=xt[S:2 * S, :, :],
                      in_=xinit[:, 8 + 4 * g:8 + 4 * g + 4, :])
        xg[g] = xt

    pending_rbc = [None, None]

    for k in range(CHUNK):
        if k % SEG == 0:
            par = (k // SEG) % 2
            # E block loads (block-diagonal corners of persistent tiles)
            for p in range(NPAIR):
                for h in range(2):
                    c = chain_of(p, h)
                    b, sub = bsub(c)
                    lt0 = sub * CHUNK + k
                    eng = dma if (p + h) % 2 == 0 else dma2
                    eng.dma_start(
                        out=etp[p][par][h * S:(h + 1) * S, :,
                                        h * S:(h + 1) * S],
                        in_=Ebuf[b, :, lt0:lt0 + SEG, :],
                    )
            # scale columns for all 16 chains on lower partitions
            s16 = p2s.tile([S, NCHAIN, SEG], F32, tag="s16")
            for c in range(NCHAIN):
                b, sub = bsub(c)
                lt0 = sub * CHUNK + k
                last_seg = (k + SEG == CHUNK)
                ncols = SEG - 1 if last_seg else SEG
                nc.vector.tensor_mul(
                    s16[:, c, 0:ncols],
                    ehat_all[:, b, lt0:lt0 + ncols],
                    invR_all[:, b, lt0 + 1:lt0 + 1 + ncols],
                )
                if last_seg:
                    nc.vector.tensor_copy(
                        s16[:, c, SEG - 1:SEG],
                        ehat_all[:, b, lt0 + SEG - 1:lt0 + SEG],
                    )
                if k == 0:
                    # chunk-start blend (no-op when alpha==1):
                    # s0 = (1 + a*(ehat0-1)) * invR_1
                    b1 = p2s.tile([S, 1], F32, tag="bl1")
                    nc.vector.tensor_scalar_add(
                        b1, ehat_all[:, b, lt0:lt0 + 1], -1.0)
                    b2 = p2s.tile([S, 1], F32, tag="bl2")
                    nc.vector.tensor_mul(b2, b1, alpha[:, c:c + 1])
                    b3 = p2s.tile([S, 1], F32, tag="bl3")
                    nc.vector.tensor_scalar_add(b3, b2, 1.0)
                    nc.vector.tensor_mul(s16[:, c, 0:1], b3,
                                         invR_all[:, b, lt0 + 1:lt0 + 2])
            if k == 0:
                # E0 = I + a*(E0 - I) on both diagonal blocks
                for p in range(NPAIR):
                    for h in range(2):
                        blk = etp[p][par][h * S:(h + 1) * S, 0,
                                          h * S:(h + 1) * S]
                        idb = identb[h * S:(h + 1) * S, h * S:(h + 1) * S]
                        av = alpha2[h * S:(h + 1) * S, p:p + 1]
                        dE = p2s.tile([2 * S, S], BF16, tag="dE")
                        nc.vector.tensor_sub(dE[h * S:(h + 1) * S, :],
                                             blk, idb)
                        dEs = p2s.tile([2 * S, S], BF16, tag="dEs")
                        nc.vector.tensor_scalar_mul(
                            dEs[h * S:(h + 1) * S, :],
                            dE[h * S:(h + 1) * S, :], av)
                        nc.vector.tensor_add(blk, dEs[h * S:(h + 1) * S, :],
                                             idb)
            # duplicate scale columns into pair layout (psum -> sbuf)
            srd_ps = p2ps.tile([2 * S, NPAIR, SEG], F32, tag="srd")
            nc.tensor.matmul(srd_ps[0:S, :, :], ident[:S, :S],
                             s16[:, 0:NPAIR, :])
            nc.tensor.matmul(srd_ps[S:2 * S, :, :], ident[:S, :S],
                             s16[:, NPAIR:NCHAIN, :])
            srd = p2s.tile([2 * S, NPAIR, SEG], F32, tag="srdsb")
            nc.vector.tensor_copy(srd, srd_ps)
            for g in range(2):
                if pending_rbc[g] is not None:
                    nc.vector.tensor_mul(srd[:, 4 * g:4 * g + 4, 0],
                                         srd[:, 4 * g:4 * g + 4, 0],
                                         pending_rbc[g][:, :])
                    pending_rbc[g] = None

        tt = k % SEG
        for g in range(2):
            ps = p2ps.tile([2 * S, 4, S], F32, tag="ps")
            for j in range(4):
                p = 4 * g + j
                nc.tensor.matmul(ps[:, j, :], etp[p][par][:, tt, :],
                                 xg[g][:, j, :])
            new_x = p2x.tile([2 * S, 4, S], BF16, tag=f"xa{g}")
            i0, i1 = bass.broadcast_tensor_aps(
                ps[:, :, :], srd[:, 4 * g:4 * g + 4, tt:tt + 1])
            nc.vector.tensor_tensor(new_x[:, :, :], i0, i1, ALU.mult)
            xg[g] = new_x

        if k % RENORM_K == RENORM_K - 1 and k < CHUNK - 1:
            slot = k // RENORM_K
            for g in range(2):
                mass = p2psr.tile([2, 4], F32, tag="mr")
                nc.tensor.matmul(mass, w["mask2"], xg[g][:, :, 0:1])
                nc.scalar.copy(msl2[:, 4 * g:4 * g + 4, slot], mass)
                minv = p2s.tile([2, 4], F32, tag="minv")
                nc.vector.reciprocal(minv, mass)
                rbc = p2psr.tile([2 * S, 4], F32, tag="rb")
                nc.tensor.matmul(rbc, w["mask2T"], minv)
                if tt + 1 < SEG:
                    nc.vector.tensor_mul(srd[:, 4 * g:4 * g + 4, tt + 1],
                                         srd[:, 4 * g:4 * g + 4, tt + 1],
                                         rbc[:, :])
                else:
                    pending_rbc[g] = rbc

    # final renorm into slot NSLOT-1 so each chunk operator has O(1) mass
    mfinal = p2m.tile([2 * S, NPAIR, S], BF16, tag="mfinal")
    for g in range(2):
        massf = p2psr.tile([2, 4], F32, tag="mr")
        nc.tensor.matmul(massf, w["mask2"], xg[g][:, :, 0:1])
        nc.scalar.copy(msl2[:, 4 * g:4 * g + 4, NSLOT - 1], massf)
        minvf = p2s.tile([2, 4], F32, tag="minv")
        nc.vector.reciprocal(minvf, massf)
        rbcf = p2psr.tile([2 * S, 4], F32, tag="rb")
        nc.tensor.matmul(rbcf, w["mask2T"], minvf)
        i0, i1 = bass.broadcast_tensor_aps(xg[g][:, :, :], rbcf[:, :, None])
        nc.vector.tensor_tensor(mfinal[:, 4 * g:4 * g + 4, :], i0, i1,
                                ALU.mult)
    return mfinal


# ======================================================================
# host side
# ======================================================================
_PROGRAM_CACHE = {}


def _get_program(has_tb3):
    key = ("nc", has_tb3)
    if key not in _PROGRAM_CACHE:
        _PROGRAM_CACHE[key] = build_program(has_tb3)
    return _PROGRAM_CACHE[key]


def host_prep(inp):
    f32 = np.float32
    bf = ml_dtypes.bfloat16
    p = {}
    tw1 = np.asarray(inp["tw1"], f32)
    p["tw1cT"] = np.ascontiguousarray(tw1[:, :C].T).astype(bf)
    p["tb1p"] = (np.asarray(inp["tb1"], f32) + tw1[:, C:].sum(1) / S).astype(f32)
    p["tw2T"] = np.ascontiguousarray(np.asarray(inp["tw2"], f32).T).astype(bf)
    p["tb2"] = np.asarray(inp["tb2"], f32)
    p["tw3T"] = np.ascontiguousarray(np.asarray(inp["tw3"], f32).T).astype(bf)
    p["tb3_bf"] = np.asarray(inp["tb3"], f32).astype(bf)
    p["fw1T_bf"] = np.ascontiguousarray(np.asarray(inp["fw1"], f32).T).astype(bf)
    p["fb1_"] = np.asarray(inp["fb1"], f32)
    p["fw2T_bf"] = np.ascontiguousarray(np.asarray(inp["fw2"], f32).T).astype(bf)
    p["fb2_"] = np.asarray(inp["fb2"], f32)
    lw = np.asarray(inp["lw"], f32)
    lb = np.asarray(inp["lb"], f32)
    p["mwT_bf"] = np.ascontiguousarray(np.asarray(inp["mw"], f32).T).astype(bf)
    p["mb_"] = np.asarray(inp["mb"], f32)
    p["lwT_bf"] = np.ascontiguousarray(lw.T).astype(bf)
    p["lb_neg"] = (-lb).astype(f32)
    p["lwsum"] = lw.sum(0).astype(bf)
    se = np.asarray(inp["state_emb"], f32)
    off_mean = se @ np.asarray(inp["mw"], f32).T
    off_lv = se @ lw.T
    E1 = np.exp(-off_lv)
    # -0.5 of the quadratic form folded into the L matrices
    p["L1"] = np.ascontiguousarray((-0.5) * E1.T)
    p["L2m"] = np.ascontiguousarray((-0.5) * (-2.0 * off_mean * E1).T)
    p["L3"] = np.ascontiguousarray((-0.5) * (off_mean**2 * E1).T)
    p["olv_bias"] = (
        -0.5 * (D * math.log(2.0 * math.pi) + off_lv.sum(1) + lb.sum())
    ).astype(f32)
    return p


def build_in_maps(inputs):
    p = host_prep(inputs)
    obs = np.asarray(inputs["observations"], np.float32)
    ctx = np.asarray(inputs["context"], np.float32).astype(ml_dtypes.bfloat16)

    in_maps = []
    for k in range(NCORES):
        t0, t1 = SLAB * k, SLAB * (k + 1)
        alpha = np.ones((S, NCHAIN), np.float32)
        if k == 0:
            alpha[:, 0:4] = 0.0   # sub-0 chains on core 0: step-0 = identity
        alpha2 = np.ones((2 * S, NCHAIN // 2), np.float32)
        alpha2[0:S, :] = alpha[:, 0:NCHAIN // 2]
        alpha2[S:2 * S, :] = alpha[:, NCHAIN // 2:]
        m2t = np.zeros((2, 2 * S), np.float32)
        m2t[0, 0:S] = 1.0
        m2t[1, S:2 * S] = 1.0
        m = {
            "obs_slab": np.ascontiguousarray(obs[:, t0:t1, :]),
            "ctx_slab": np.ascontiguousarray(ctx[:, t0:t1, :]),
            "alpha_blend": alpha,
            "alpha_blend2": alpha2,
            "mask2T_": m2t,
        }
        m.update(p)
        in_maps.append(m)
    return in_maps


def kernel(**inputs):
    has_tb3 = bool(np.any(np.asarray(inputs["tb3"]) != 0))
    nc = _get_program(has_tb3)
    in_maps = build_in_maps(inputs)
    res = run_bass_kernel_spmd(nc, in_maps, core_ids=list(range(NCORES)))
    return np.asarray(res.results[0]["ll_out"], np.float32)


if __name__ == "__main__":
    sys.path.insert(0, "/root/problem")
    import reference

    inp = {k: np.asarray(v) for k, v in reference.setup_inputs().items()}
    got = kernel(**inp)
    print("kernel:", got)


# revision 45
# speedup vs baseline: 2.2608x; 1.1923x over previous
"""NeuralHMM forward log-likelihood on 8 Trainium2 NeuronCores.

Strategy (data-parallel over time slabs, associative chunk combine):
  - Core k owns time slab t in [256k, 256(k+1)) for ALL batch elements.
  - Phase 1 (parallel over (b,t)): transition MLP -> E = exp(logits) (bf16)
    written to DRAM in [b][i][t][j] layout (i = from-state) so that the
    phase-2 chain loads are contiguous per partition; per-row sums R ->
    invR and obs-model ehat = exp(obs_lp - n) both kept resident in SBUF
    as [state][b][t]; n (per-t normalizer) accumulated into nu.
  - Phase 2: linear-domain operator chains. Per (b, half-slab) chunk of
    128 steps:  M = prod_t D(s_t) E_t^T,  s_t = ehat_t * invR_{t+1} (last
    step of a chunk uses ehat only; chunk init is D(invR_lo)).  8 chunks
    per core run interleaved; the 8 per-step scale ops are fused into ONE
    DVE instruction via a stride-0 broadcast of the packed scale column.
    Renorm every 32 steps for all chains jointly, with exact log
    accounting.
  - Each core pre-combines its two chunk operators per b (P_b = M_hi M_lo),
    AllGathers the 4 P matrices + scalars, then every core redundantly
    runs the 8-step operator/vector chain:
      ll_b = log(1^T P_7 ... P_0 a0) + sum_t n(b,t) + sum(renorm logs)
             - log(S).
Weight-only reshapes/transposes are precomputed on host in kernel().
"""

import math
import os
import sys

import numpy as np

BUILD_STAGE = int(os.environ.get("NHMM_STAGE", "4"))  # 1=p1, 2=+chains, 3=+cc, 4=all
P1_STAGE = int(os.environ.get("NHMM_P1", "4"))  # 1=loads, 2=+mlp, 3=+logits, 4=all
DEBUG = os.environ.get("NHMM_DEBUG", "0") == "1"

sys.path.insert(0, "/opt/trn_rl_repo")

import ml_dtypes  # noqa: E402

import concourse.bass as bass  # noqa: E402
import concourse.tile as tile  # noqa: E402
from concourse import bacc, mybir  # noqa: E402
from concourse.bass_utils import run_bass_kernel_spmd  # noqa: E402
from concourse.masks import make_identity  # noqa: E402

F32 = mybir.dt.float32
BF16 = mybir.dt.bfloat16
AF = mybir.ActivationFunctionType
AX = mybir.AxisListType
ALU = mybir.AluOpType

B, T, D = 4, 2048, 80
S, H, C = 64, 256, 128
NCORES = 8
SLAB = T // NCORES        # 256 timesteps per core
NTILE = 128               # phase-1 tile width
NT_TILES = SLAB // NTILE  # 2
NCHAIN = 16               # interleaved operator chains per core
NPAIR = NCHAIN // 2       # chains pair-packed into 128 partitions
CHUNK = SLAB // 4         # 64 steps per chain
SEG = 16                  # steps per E-load segment
NSEG = CHUNK // SEG       # 4 segments per chunk
RENORM_K = 32
NSLOT = 2                 # renorm at k=31 + final normalize
LSLICE = 512

CONTRIB = B * S * S + B + NCHAIN + B * S
OFF_M = 0
OFF_NU = B * S * S
OFF_LAM = OFF_NU + B
OFF_A0 = OFF_LAM + NCHAIN


def build_program(has_tb3):
    nc = bacc.Bacc(
        "TRN2",
        target_bir_lowering=False,
        debug=False,
        enable_asserts=False,
        num_devices=NCORES,
    )

    def din(name, shape, dtype=F32):
        return nc.dram_tensor(name, list(shape), dtype, kind="ExternalInput").ap()

    io = {}
    io["obs"] = din("obs_slab", (B, SLAB, D))
    io["ctx"] = din("ctx_slab", (B, SLAB, C), BF16)
    io["alpha"] = din("alpha_blend", (S, NCHAIN))
    io["alpha2"] = din("alpha_blend2", (2 * S, NPAIR))
    io["mask2T"] = din("mask2T_", (2, 2 * S))
    io["tw1cT"] = din("tw1cT", (C, H), BF16)
    io["tb1p"] = din("tb1p", (H,))
    io["tw2T"] = din("tw2T", (H, H), BF16)
    io["tb2"] = din("tb2", (H,))
    io["tw3T"] = din("tw3T", (H, S * S), BF16)
    io["tb3"] = din("tb3_bf", (S * S,), BF16)
    io["fw1T"] = din("fw1T_bf", (D, H), BF16)
    io["fb1"] = din("fb1_", (H,))
    io["fw2T"] = din("fw2T_bf", (H, H), BF16)
    io["fb2"] = din("fb2_", (H,))
    io["mwT"] = din("mwT_bf", (H, D), BF16)
    io["mb"] = din("mb_", (D,))
    io["lwT"] = din("lwT_bf", (H, D), BF16)
    io["lbn"] = din("lb_neg", (D,))
    io["lwsum"] = din("lwsum", (H,), BF16)
    io["L1"] = din("L1", (D, S))      # -0.5 folded in on host
    io["L2m"] = din("L2m", (D, S))
    io["L3"] = din("L3", (D, S))
    io["olv_bias"] = din("olv_bias", (S,))

    io["out"] = nc.dram_tensor("ll_out", [B], F32, kind="ExternalOutput").ap()
    io["Ebuf"] = nc.dram_tensor("Ebuf", [B, S, SLAB, S], BF16).ap()
    io["contrib"] = nc.dram_tensor("contrib", [CONTRIB], BF16).ap()
    if DEBUG:
        io["Ebuf_dbg"] = nc.dram_tensor(
            "Ebuf_dbg", [B, S, SLAB, S], BF16, kind="ExternalOutput").ap()
        io["contrib_dbg"] = nc.dram_tensor(
            "contrib_dbg", [CONTRIB], BF16, kind="ExternalOutput").ap()
        io["ehat_dbg"] = nc.dram_tensor(
            "ehat_dbg", [S, B, SLAB], F32, kind="ExternalOutput").ap()
        io["invR_dbg"] = nc.dram_tensor(
            "invR_dbg", [S, B, SLAB], F32, kind="ExternalOutput").ap()
        io["gath_dbg"] = nc.dram_tensor(
            "gath_dbg", [NCORES * CONTRIB], BF16, kind="ExternalOutput").ap()
        io["cmb_dbg"] = nc.dram_tensor(
            "cmb_dbg", [B, 8], F32, kind="ExternalOutput").ap()
    io["gathered"] = nc.dram_tensor(
        "gathered", [NCORES * CONTRIB], BF16, addr_space="Shared"
    ).ap()

    with tile.TileContext(nc) as tc:
        build_tile(tc, io, has_tb3)
    nc.compile()
    return nc


def build_tile(tc, io, has_tb3):
    nc = tc.nc
    dma = nc.sync
    dma2 = nc.scalar
    with (
        tc.tile_pool(name="wts", bufs=1) as wts,
        tc.tile_pool(name="consts", bufs=1) as consts,
        tc.tile_pool(name="p2w", bufs=1) as p2w,
    ):
        ident = consts.tile([128, 128], F32)
        make_identity(nc, ident)
        identb = consts.tile([128, 128], BF16)
        nc.vector.tensor_copy(identb, ident)

        def load_w(tag, ap_dram, shape, dtype=F32, eng=dma):
            t = wts.tile(list(shape), dtype, tag=tag)
            eng.dma_start(out=t, in_=ap_dram)
            return t

        w = {}
        w["tw1cT"] = load_w("w1", io["tw1cT"], (C, H), BF16)
        w["tw2T"] = [load_w(f"w2{k}", io["tw2T"][k * 128:(k + 1) * 128, :],
                            (128, H), BF16) for k in range(2)]
        w["tw3T"] = [load_w(f"w3{k}", io["tw3T"][k * 128:(k + 1) * 128, :],
                            (128, S * S), BF16, eng=dma2) for k in range(2)]
        if has_tb3:
            w["tb3"] = load_w("b3", io["tb3"][None, :], (1, S * S), BF16)
        w["fw1T"] = load_w("g1", io["fw1T"], (D, H), BF16)
        w["fw2T"] = [load_w(f"g2{k}", io["fw2T"][k * 128:(k + 1) * 128, :],
                            (128, H), BF16) for k in range(2)]
        w["mwT"] = [load_w(f"gm{k}", io["mwT"][k * 128:(k + 1) * 128, :],
                           (128, D), BF16) for k in range(2)]
        w["lwT"] = [load_w(f"gl{k}", io["lwT"][k * 128:(k + 1) * 128, :],
                           (128, D), BF16) for k in range(2)]
        w["L1"] = load_w("L1", io["L1"], (D, S))
        w["L2m"] = load_w("L2m", io["L2m"], (D, S))
        w["L3"] = load_w("L3", io["L3"], (D, S))
        for nm in ("tb1p", "tb2", "fb1", "fb2"):
            w[nm] = [load_w(f"{nm}{k}", io[nm][k * 128:(k + 1) * 128, None],
                            (128, 1)) for k in range(2)]
        w["lwsum"] = [load_w(f"lws{k}", io["lwsum"][k * 128:(k + 1) * 128, None],
                             (128, 1), BF16) for k in range(2)]
        w["mb"] = load_w("mb", io["mb"][:, None], (D, 1))
        w["lbn"] = load_w("lbn", io["lbn"][:, None], (D, 1))
        w["olvb"] = load_w("olvb", io["olv_bias"][:, None], (S, 1))
        w["alpha"] = load_w("alpha", io["alpha"], (S, NCHAIN))
        w["alpha2"] = load_w("alpha2", io["alpha2"], (2 * S, NPAIR))

        ones_bt = consts.tile([1, NTILE], BF16)      # lhsT for tb3 rank-1
        nc.vector.memset(ones_bt, 1.0)
        pones_row = consts.tile([1, S], F32)         # +1 lhsT (bcast via PE)
        nc.vector.memset(pones_row, 1.0)
        nones_row = consts.tile([1, S], F32)         # -1 lhsT (mean bcast)
        nc.vector.memset(nones_row, -1.0)
        invS_col = consts.tile([S, 1], F32)
        nc.vector.memset(invS_col, 1.0 / S)
        ones_colb = consts.tile([S, 1], BF16)        # mass lhsT (bf16)
        nc.vector.memset(ones_colb, 1.0)
        ones_colf = consts.tile([S, 1], F32)         # mass lhsT (f32)
        nc.vector.memset(ones_colf, 1.0)
        mask2 = consts.tile([2 * S, 2], BF16)        # per-half mass lhsT
        nc.vector.memset(mask2, 0.0)
        nc.vector.memset(mask2[0:S, 0:1], 1.0)
        nc.vector.memset(mask2[S:2 * S, 1:2], 1.0)
        mask2T = load_w("mask2T", io["mask2T"], (2, 2 * S))
        w.update(ones_bt=ones_bt, pones_row=pones_row, nones_row=nones_row,
                 invS_col=invS_col, ones_colb=ones_colb, ones_colf=ones_colf,
                 mask2=mask2, mask2T=mask2T, ident=ident, identb=identb)

        # SBUF-resident per-(state, b, t) tensors shared across phases
        ehat_all = consts.tile([S, B, SLAB], F32)
        invR_all = consts.tile([S, B, SLAB], F32)
        nu_sb = consts.tile([1, B], F32)
        lam2 = consts.tile([2, NPAIR], F32)
        msl2 = consts.tile([2, NPAIR, NSLOT], F32)
        nc.vector.memset(msl2, 1.0)
        # persistent pair-packed E tiles (block-diagonal lhsT), double-buffered
        etp = [[None, None] for _ in range(NPAIR)]
        for p in range(NPAIR):
            for par in range(2):
                t = p2w.tile([2 * S, SEG, 2 * S], BF16, tag=f"etp{p}_{par}")
                nc.gpsimd.memset(t, 0.0)
                etp[p][par] = t
        nslots_n = consts.tile([1, B, NT_TILES], F32)
        nslots_s = consts.tile([1, B, NT_TILES], F32)

        # ================= PHASE 1 =================
        with (
            tc.tile_pool(name="p1", bufs=2) as p1,
            tc.tile_pool(name="pbig", bufs=3, space="PSUM") as pbig,
            tc.tile_pool(name="plg", bufs=2, space="PSUM") as plg,
            tc.tile_pool(name="psm", bufs=3, space="PSUM") as psm,
        ):
            for hh in range(NT_TILES):
                for b in range(B):
                    phase1_tile(nc, b, hh, dma, dma2, p1, pbig, plg, psm,
                                w, io, ehat_all, invR_all, nslots_n, nslots_s,
                                has_tb3)
            if P1_STAGE >= 4:
                for b in range(B):
                    t1 = p1.tile([1, 1], F32, tag="nut1")
                    nc.vector.reduce_sum(t1, nslots_n[:, b, :], axis=AX.X)
                    t2 = p1.tile([1, 1], F32, tag="nut2")
                    nc.vector.reduce_sum(t2, nslots_s[:, b, :], axis=AX.X)
                    t3 = p1.tile([1, 1], F32, tag="nut3")
                    nc.vector.tensor_scalar_mul(t3, t2, -0.5)
                    nc.vector.tensor_add(nu_sb[:, b:b + 1], t1, t3)
            if DEBUG:
                dma.dma_start(out=io["ehat_dbg"], in_=ehat_all)
                dma.dma_start(out=io["invR_dbg"], in_=invR_all)
                dma.dma_start(out=io["Ebuf_dbg"], in_=io["Ebuf"])

        if BUILD_STAGE < 2:
            return
        # ================= PHASE 2 =================
        with tc.tile_pool(name="p2m", bufs=1) as p2m:
            with (
                tc.tile_pool(name="p2s", bufs=2) as p2s,
                tc.tile_pool(name="p2x", bufs=2) as p2x,
                tc.tile_pool(name="p2ps", bufs=2, space="PSUM") as p2ps,
                tc.tile_pool(name="p2psr", bufs=1, space="PSUM") as p2psr,
            ):
                mfinal = run_chains(nc, dma, dma2, p2s, p2x, p2m, p2ps,
                                    p2psr, w, io, msl2, ehat_all, invR_all,
                                    etp)

                lnms = p2m.tile([2, NPAIR, NSLOT], F32)
                nc.scalar.activation(lnms, msl2, AF.Ln)
                for p in range(NPAIR):
                    nc.vector.reduce_sum(lam2[:, p:p + 1], lnms[:, p, :],
                                         axis=AX.X)

            if BUILD_STAGE < 3:
                return
            # pre-combine the core's four chunks per b into V_b = P_b^T,
            # P_b = M3 M2 M1 M0  =>  V_b = M0^T M1^T M2^T M3^T
            contrib = io["contrib"]
            with tc.tile_pool(name="pcps", bufs=2, space="PSUM") as pcps:
                pstage = p2m.tile([S, B, S], BF16, tag="pstage")
                for b in range(B):
                    # chunk operators: sub0 = (lo, b), sub1 = (lo, 4+b),
                    # sub2 = (up, b), sub3 = (up, 4+b) in mfinal [128, 8, 64]
                    m0 = mfinal[0:S, b, :]
                    m1 = mfinal[0:S, 4 + b, :]
                    m2 = mfinal[S:2 * S, b, :]
                    m3 = mfinal[S:2 * S, 4 + b, :]
                    m3t_ps = pcps.tile([2 * S, S], BF16, tag="xt")
                    nc.tensor.transpose(m3t_ps[S:2 * S, :], m3,
                                        identb[S:2 * S, S:2 * S])
                    m3t = p2m.tile([2 * S, S], BF16, tag=f"m3t{b}")
                    nc.vector.tensor_copy(m3t[S:2 * S, :], m3t_ps[S:2 * S, :])
                    z1_ps = pcps.tile([S, S], F32, tag="pp")
                    nc.tensor.matmul(z1_ps, m2, m3t[S:2 * S, :])
                    z1 = p2m.tile([S, S], BF16, tag=f"z1{b}")
                    nc.vector.tensor_copy(z1, z1_ps)
                    z2_ps = pcps.tile([S, S], F32, tag="pp")
                    nc.tensor.matmul(z2_ps, m1, z1)
                    z2 = p2m.tile([S, S], BF16, tag=f"z2{b}")
                    nc.vector.tensor_copy(z2, z2_ps)
                    v_ps = pcps.tile([S, S], F32, tag="pp")
                    nc.tensor.matmul(v_ps, m0, z2)
                    nc.scalar.copy(pstage[:, b, :], v_ps)
                dma.dma_start(
                    out=contrib[OFF_M:OFF_M + B * S * S]
                    .rearrange("(b2 a c) -> a b2 c", b2=B, a=S),
                    in_=pstage,
                )
                nc.gpsimd.dma_start(out=contrib[OFF_NU:OFF_NU + B][None, :],
                                    in_=nu_sb)
                nc.gpsimd.dma_start(
                    out=contrib[OFF_LAM:OFF_LAM + NCHAIN]
                    .rearrange("(h p) -> h p", h=2),
                    in_=lam2,
                )
                nc.gpsimd.dma_start(
                    out=contrib[OFF_A0:OFF_A0 + B * S]
                    .rearrange("(s b2) -> s b2", s=S),
                    in_=ehat_all[:, :, 0],
                )
            if DEBUG:
                dma.dma_start(out=io["contrib_dbg"], in_=contrib)
            with tc.tile_critical():
                with nc.semaphore("ccsem") as ccsem:
                    nc.gpsimd.collective_compute(
                        "AllGather",
                        ALU.bypass,
                        replica_groups=[list(range(NCORES))],
                        ins=[contrib],
                        outs=[io["gathered"]],
                    ).then_inc(ccsem, 1)
                    nc.gpsimd.wait_ge(ccsem, 1)

        if BUILD_STAGE < 4:
            return
        # ================= COMBINE =================
        with (
            tc.tile_pool(name="cmb", bufs=2) as cmb,
            tc.tile_pool(name="cmbps", bufs=2, space="PSUM") as cmbps,
        ):
            g2 = io["gathered"].rearrange("(k f) -> k f", k=NCORES)
            if DEBUG:
                dma.dma_start(out=io["gath_dbg"], in_=io["gathered"])
            cdbg = None
            if DEBUG:
                cdbg = cmb.tile([1, B, 8], F32, tag="cdbg")
            out_row = cmb.tile([1, B], F32, tag="outrow")
            # one gathered load of all per-core scalars; fp32 core-sums
            # via a ones-lhsT matmul (contraction over the core axis)
            NS = B + NCHAIN
            sc_sb = cmb.tile([NCORES, NS], BF16, tag="scsb")
            dma.dma_start(out=sc_sb, in_=g2[:, OFF_NU:OFF_NU + NS])
            ones8 = cmb.tile([NCORES, 1], BF16, tag="ones8")
            nc.vector.memset(ones8, 1.0)
            ssum_ps = cmbps.tile([1, NS], F32, tag="ssum")
            nc.tensor.matmul(ssum_ps, ones8, sc_sb)
            ssum = cmb.tile([1, NS], F32, tag="ssums")
            nc.scalar.copy(ssum, ssum_ps)
            for b in range(B):
                w_ = cmb.tile([S, 1], BF16, tag=f"u{b}")
                dma2.dma_start(
                    out=w_,
                    in_=g2[0, OFF_A0:OFF_A0 + B * S]
                    .rearrange("(s b2) -> s b2", s=S)[:, b:b + 1],
                )
                m_all = cmb.tile([S, NCORES, S], BF16, tag=f"m{b}")
                eng = dma if b % 2 == 0 else dma2
                eng.dma_start(
                    out=m_all,
                    in_=g2[:, OFF_M + b * S * S:OFF_M + (b + 1) * S * S]
                    .rearrange("k (a c) -> a k c", a=S),
                )
                for r in range(NCORES):
                    up = cmbps.tile([S, 1], F32, tag="up")
                    nc.tensor.matmul(up, m_all[:, r, :], w_)
                    w_ = cmb.tile([S, 1], BF16, tag=f"u{b}")
                    nc.vector.tensor_copy(w_, up)
                dotp = cmbps.tile([1, 1], F32, tag="dot")
                nc.tensor.matmul(dotp, w["ones_colb"], w_)

                lamsum = cmb.tile([1, 1], F32, tag=f"lams{b}")
                nc.vector.reduce_sum(
                    lamsum, ssum[:, B + b:B + NCHAIN:4], axis=AX.X)
                lnv = cmb.tile([1, 1], F32, tag=f"lnv{b}")
                nc.scalar.activation(lnv, dotp, AF.Ln)
                acc = cmb.tile([1, 1], F32, tag=f"acc{b}")
                nc.vector.tensor_add(acc, lnv, ssum[:, b:b + 1])
                acc2 = cmb.tile([1, 1], F32, tag=f"acc2{b}")
                nc.vector.tensor_add(acc2, acc, lamsum)
                nc.vector.tensor_scalar_add(out_row[:, b:b + 1], acc2,
                                            -math.log(S))
                if DEBUG:
                    nc.scalar.copy(cdbg[:, b, 0:1], lnv)
                    nc.scalar.copy(cdbg[:, b, 1:2], lnv)
                    nc.scalar.copy(cdbg[:, b, 2:3], ssum[:, b:b + 1])
                    nc.scalar.copy(cdbg[:, b, 3:4], lamsum)
                    nc.scalar.copy(cdbg[:, b, 4:5], dotp)
                    nc.scalar.copy(cdbg[:, b, 5:6], dotp)
                    nc.scalar.copy(cdbg[:, b, 6:7], w_[0:1, :])
                    nc.scalar.copy(cdbg[:, b, 7:8], w_[0:1, :])
            if DEBUG:
                dma.dma_start(out=io["cmb_dbg"][None, :, :], in_=cdbg)
            dma.dma_start(out=io["out"][None, :], in_=out_row)


def phase1_tile(nc, b, hh, dma, dma2, p1, pbig, plg, psm, w, io,
                ehat_all, invR_all, nslots_n, nslots_s, has_tb3):
    nt = NTILE
    t0 = hh * NTILE
    ident = w["ident"]

    ctx_t = p1.tile([nt, C], BF16, tag="ctxt")
    dma.dma_start(out=ctx_t, in_=io["ctx"][b, t0:t0 + nt, :])
    obs_t = p1.tile([nt, D], F32, tag="obst")
    dma2.dma_start(out=obs_t, in_=io["obs"][b, t0:t0 + nt, :])

    ctxT_ps = pbig.tile([C, nt], BF16, tag="pp")
    nc.tensor.transpose(ctxT_ps, ctx_t, w["identb"])
    ctxT = p1.tile([C, nt], BF16, tag="ctxT")
    nc.vector.tensor_copy(ctxT, ctxT_ps)

    obsT_ps = psm.tile([D, nt], F32, tag="sm")
    nc.tensor.transpose(obsT_ps, obs_t, ident)
    obsT = p1.tile([D, nt], F32, tag="obsT")
    nc.vector.tensor_copy(obsT, obsT_ps)
    obsT_bf = p1.tile([D, nt], BF16, tag="obsTb")
    nc.scalar.copy(obsT_bf, obsT_ps)

    if P1_STAGE < 2:
        return
    # transition MLP (feature-on-partition, bf16)
    h1 = []
    for m in range(2):
        ps = pbig.tile([128, nt], F32, tag="pp")
        nc.tensor.matmul(ps, w["tw1cT"][:, m * 128:(m + 1) * 128], ctxT)
        sb = p1.tile([128, nt], BF16, tag=f"h1_{m}")
        nc.scalar.activation(sb, ps, AF.Relu, bias=w["tb1p"][m], scale=1.0)
        h1.append(sb)
    h2 = []
    for m in range(2):
        ps = pbig.tile([128, nt], F32, tag="pp")
        for k in range(2):
            nc.tensor.matmul(ps, w["tw2T"][k][:, m * 128:(m + 1) * 128], h1[k],
                             start=(k == 0), stop=(k == 1))
        sb = p1.tile([128, nt], BF16, tag=f"h2_{m}")
        nc.scalar.activation(sb, ps, AF.Relu, bias=w["tb2"][m], scale=1.0)
        h2.append(sb)

    if P1_STAGE < 3:
        return
    # logits slices -> exp (bf16) -> esl_all; R accumulation; E write
    esl_all = p1.tile([nt, S * S], BF16, tag="esl")
    R_sb = p1.tile([nt, S], BF16, tag="Rsb")
    nsl = (S * S) // LSLICE
    for sl in range(nsl):
        ps = plg.tile([nt, LSLICE], F32, tag="lgp")
        last = not has_tb3
        for k in range(2):
            nc.tensor.matmul(ps, h2[k],
                             w["tw3T"][k][:, sl * LSLICE:(sl + 1) * LSLICE],
                             start=(k == 0), stop=(k == 1) and last)
        if has_tb3:
            nc.tensor.matmul(ps, w["ones_bt"],
                             w["tb3"][:, sl * LSLICE:(sl + 1) * LSLICE],
                             start=False, stop=True)
        esl = esl_all[:, sl * LSLICE:(sl + 1) * LSLICE]
        nc.scalar.activation(esl, ps, AF.Exp)
        ni = LSLICE // S
        with nc.allow_low_precision(reason="bf16 R row-sum, ~0.5% rel ok"):
            nc.vector.reduce_sum(
                R_sb[:, sl * ni:(sl + 1) * ni],
                esl.rearrange("p (i j) -> p i j", j=S),
                axis=AX.X,
            )
    dma.dma_start(
        out=io["Ebuf"][b, :, t0:t0 + nt, :].rearrange("i t j -> t i j"),
        in_=esl_all.rearrange("t (i j) -> t i j", i=S),
    )

    RT_ps = psm.tile([S, nt], BF16, tag="sm")
    nc.tensor.transpose(RT_ps, R_sb, w["identb"])
    nc.vector.reciprocal(invR_all[:, b, t0:t0 + nt], RT_ps)

    if P1_STAGE < 4:
        return
    # observation model (bf16 matmuls, fp32 quadratic form)
    f1 = []
    for m in range(2):
        ps = pbig.tile([128, nt], F32, tag="pp")
        nc.tensor.matmul(ps, w["fw1T"][:, m * 128:(m + 1) * 128], obsT_bf)
        sb = p1.tile([128, nt], BF16, tag=f"f1_{m}")
        nc.vector.tensor_scalar(sb, ps, w["fb1"][m], 0.0, ALU.add, ALU.max)
        f1.append(sb)
    f2 = []
    for m in range(2):
        ps = pbig.tile([128, nt], F32, tag="pp")
        for k in range(2):
            nc.tensor.matmul(ps, w["fw2T"][k][:, m * 128:(m + 1) * 128], f1[k],
                             start=(k == 0), stop=(k == 1))
        sb = p1.tile([128, nt], BF16, tag=f"f2_{m}")
        nc.vector.tensor_scalar(sb, ps, w["fb2"][m], 0.0, ALU.add, ALU.max)
        f2.append(sb)

    bm_ps = psm.tile([D, nt], F32, tag="sm")
    for k in range(2):
        nc.tensor.matmul(bm_ps, w["mwT"][k], f2[k], start=(k == 0), stop=(k == 1))
    blv_ps = psm.tile([D, nt], F32, tag="sm")
    for k in range(2):
        nc.tensor.matmul(blv_ps, w["lwT"][k], f2[k], start=(k == 0), stop=(k == 1))
    sblv_ps = psm.tile([1, nt], F32, tag="sm")
    for k in range(2):
        nc.tensor.matmul(sblv_ps, w["lwsum"][k], f2[k],
                         start=(k == 0), stop=(k == 1))
    nc.vector.reduce_sum(nslots_s[:, b, hh:hh + 1], sblv_ps, axis=AX.X)

    # e_ = exp(-(blv + lb));  r_ = (obs - mb) - bm
    e_ = p1.tile([D, nt], F32, tag="e_")
    nc.scalar.activation(e_, blv_ps, AF.Exp, bias=w["lbn"], scale=-1.0)
    obs2 = p1.tile([D, nt], F32, tag="obs2")
    nc.vector.tensor_scalar_sub(obs2, obsT, w["mb"])
    r_ = p1.tile([D, nt], F32, tag="r_")
    nc.vector.tensor_sub(r_, obs2, bm_ps)
    Bm_ = p1.tile([D, nt], F32, tag="Bm_")
    nc.vector.tensor_mul(Bm_, r_, e_)
    A_ = p1.tile([D, nt], F32, tag="A_")
    nc.vector.tensor_mul(A_, r_, Bm_)

    q_ps = psm.tile([S, nt], F32, tag="sm")
    nc.tensor.matmul(q_ps, w["L1"], A_, start=True, stop=False)
    nc.tensor.matmul(q_ps, w["L2m"], Bm_, start=False, stop=False)
    nc.tensor.matmul(q_ps, w["L3"], e_, start=False, stop=True)

    lp0 = p1.tile([S, nt], F32, tag="lp0")
    nc.scalar.activation(lp0, q_ps, AF.Identity, bias=w["olvb"], scale=1.0)

    n_ps = psm.tile([1, nt], F32, tag="sm")
    nc.tensor.matmul(n_ps, w["invS_col"], lp0)
    n_sb = p1.tile([1, nt], F32, tag="nsb")
    nc.scalar.copy(n_sb, n_ps)
    d_ps = psm.tile([S, nt], F32, tag="sm")
    nc.tensor.matmul(d_ps, w["nones_row"], n_sb)      # = -n broadcast
    dd = p1.tile([S, nt], F32, tag="dd")
    nc.vector.tensor_add(dd, lp0, d_ps)
    nc.scalar.activation(ehat_all[:, b, t0:t0 + nt], dd, AF.Exp)

    nc.vector.reduce_sum(nslots_n[:, b, hh:hh + 1], n_sb, axis=AX.X)
    nc.vector.reduce_sum(nslots_s[:, b, hh:hh + 1], sblv_ps, axis=AX.X)


def run_chains(nc, dma, dma2, p2s, p2x, p2m, p2ps, p2psr, w, io,
               msl2, ehat_all, invR_all, etp):
    """16 chains, pair-packed 2-per-128-partitions; chain c handles
    (b = c%4, sub = c//4), chunk = local t in [sub*64, sub*64+64).
    Pair p stacks chain p (partitions 0:64) and chain 8+p (64:128) with
    block-diagonal E tiles; groups g in {0,1} cover pairs 4g..4g+3."""
    ident = w["ident"]
    identb = w["identb"]
    alpha = w["alpha"]
    alpha2 = w["alpha2"]
    Ebuf = io["Ebuf"]

    def chain_of(p, h):
        return h * NPAIR + p

    def bsub(c):
        return c % 4, c // 4

    # per-chain diag init on lower partitions, then DMA into pair layout
    xinit = p2s.tile([S, NCHAIN, S], BF16, tag="xinit")
    for c in range(NCHAIN):
        b, sub = bsub(c)
        lo = sub * CHUNK
        t1 = p2s.tile([S, 1], F32, tag="ir0a")
        nc.vector.tensor_scalar_add(t1, invR_all[:, b, lo:lo + 1], -1.0)
        t2 = p2s.tile([S, 1], F32, tag="ir0b")
        nc.vector.tensor_mul(t2, t1, alpha[:, c:c + 1])
        t3 = p2s.tile([S, 1], F32, tag="ir0c")
        nc.vector.tensor_scalar_add(t3, t2, 1.0)
        nc.vector.tensor_scalar_mul(xinit[:, c, :], ident[:S, :S], t3)
    xg = [None, None]
    for g in range(2):
        xt = p2x.tile([2 * S, 4, S], BF16, tag=f"xa{g}")
        dma.dma_start(out=xt[0:S, :, :], in_=xinit[:, 4 * g:4 * g + 4, :])
        dma.dma_start(out=xt[S:2 * S, :, :],
                      in_=xinit[:, 8 + 4 * g:8 + 4 * g + 4, :])
        xg[g] = xt

    pending_rbc = [None, None]

    for k in range(CHUNK):
        if k % SEG == 0:
            par = (k // SEG) % 2
            # E block loads (block-diagonal corners of persistent tiles)
            for p in range(NPAIR):
                for h in range(2):
                    c = chain_of(p, h)
                    b, sub = bsub(c)
                    lt0 = sub * CHUNK + k
                    eng = dma if (p + h) % 2 == 0 else dma2
                    eng.dma_start(
                        out=etp[p][par][h * S:(h + 1) * S, :,
                                        h * S:(h + 1) * S],
                        in_=Ebuf[b, :, lt0:lt0 + SEG, :],
                    )
            # scale columns for all 16 chains on lower partitions
            s16 = p2s.tile([S, NCHAIN, SEG], F32, tag="s16")
            for c in range(NCHAIN):
                b, sub = bsub(c)
                lt0 = sub * CHUNK + k
                last_seg = (k + SEG == CHUNK)
                ncols = SEG - 1 if last_seg else SEG
                nc.vector.tensor_mul(
                    s16[:, c, 0:ncols],
                    ehat_all[:, b, lt0:lt0 + ncols],
                    invR_all[:, b, lt0 + 1:lt0 + 1 + ncols],
                )
                if last_seg:
                    nc.vector.tensor_copy(
                        s16[:, c, SEG - 1:SEG],
                        ehat_all[:, b, lt0 + SEG - 1:lt0 + SEG],
                    )
                if k == 0:
                    # chunk-start blend (no-op when alpha==1):
                    # s0 = (1 + a*(ehat0-1)) * invR_1
                    b1 = p2s.tile([S, 1], F32, tag="bl1")
                    nc.vector.tensor_scalar_add(
                        b1, ehat_all[:, b, lt0:lt0 + 1], -1.0)
                    b2 = p2s.tile([S, 1], F32, tag="bl2")
                    nc.vector.tensor_mul(b2, b1, alpha[:, c:c + 1])
                    b3 = p2s.tile([S, 1], F32, tag="bl3")
                    nc.vector.tensor_scalar_add(b3, b2, 1.0)
                    nc.vector.tensor_mul(s16[:, c, 0:1], b3,
                                         invR_all[:, b, lt0 + 1:lt0 + 2])
            if k == 0:
                # E0 = I + a*(E0 - I) on both diagonal blocks
                for p in range(NPAIR):
                    for h in range(2):
                        blk = etp[p][par][h * S:(h + 1) * S, 0,
                                          h * S:(h + 1) * S]
                        idb = identb[h * S:(h + 1) * S, h * S:(h + 1) * S]
                        av = alpha2[h * S:(h + 1) * S, p:p + 1]
                        dE = p2s.tile([2 * S, S], BF16, tag="dE")
                        nc.vector.tensor_sub(dE[h * S:(h + 1) * S, :],
                                             blk, idb)
                        dEs = p2s.tile([2 * S, S], BF16, tag="dEs")
                        nc.vector.tensor_scalar_mul(
                            dEs[h * S:(h + 1) * S, :],
                            dE[h * S:(h + 1) * S, :], av)
                        nc.vector.tensor_add(blk, dEs[h * S:(h + 1) * S, :],
                                             idb)
            # duplicate scale columns into pair layout (psum -> sbuf)
            srd_ps = p2ps.tile([2 * S, NPAIR, SEG], F32, tag="srd")
            nc.tensor.matmul(srd_ps[0:S, :, :], ident[:S, :S],
                             s16[:, 0:NPAIR, :])
            nc.tensor.matmul(srd_ps[S:2 * S, :, :], ident[:S, :S],
                             s16[:, NPAIR:NCHAIN, :])
            srd = p2s.tile([2 * S, NPAIR, SEG], F32, tag="srdsb")
            nc.vector.tensor_copy(srd, srd_ps)
            for g in range(2):
                if pending_rbc[g] is not None:
                    nc.vector.tensor_mul(srd[:, 4 * g:4 * g + 4, 0],
                                         srd[:, 4 * g:4 * g + 4, 0],
                                         pending_rbc[g][:, :])
                    pending_rbc[g] = None

        tt = k % SEG
        for g in range(2):
            ps = p2ps.tile([2 * S, 4, S], F32, tag="ps")
            for j in range(4):
                p = 4 * g + j
                nc.tensor.matmul(ps[:, j, :], etp[p][par][:, tt, :],
                                 xg[g][:, j, :])
            new_x = p2x.tile([2 * S, 4, S], BF16, tag=f"xa{g}")
            i0, i1 = bass.broadcast_tensor_aps(
                ps[:, :, :], srd[:, 4 * g:4 * g + 4, tt:tt + 1])
            nc.vector.tensor_tensor(new_x[:, :, :], i0, i1, ALU.mult)
            xg[g] = new_x

        if k % RENORM_K == RENORM_K - 1 and k < CHUNK - 1:
            slot = k // RENORM_K
            for g in range(2):
                mass = p2psr.tile([2, 4], F32, tag="mr")
                nc.tensor.matmul(mass, w["mask2"], xg[g][:, :, 0:1])
                nc.scalar.copy(msl2[:, 4 * g:4 * g + 4, slot], mass)
                minv = p2s.tile([2, 4], F32, tag="minv")
                nc.vector.reciprocal(minv, mass)
                rbc = p2psr.tile([2 * S, 4], F32, tag="rb")
                nc.tensor.matmul(rbc, w["mask2T"], minv)
                if tt + 1 < SEG:
                    nc.vector.tensor_mul(srd[:, 4 * g:4 * g + 4, tt + 1],
                                         srd[:, 4 * g:4 * g + 4, tt + 1],
                                         rbc[:, :])
                else:
                    pending_rbc[g] = rbc

    # final renorm into slot NSLOT-1 so each chunk operator has O(1) mass
    mfinal = p2m.tile([2 * S, NPAIR, S], BF16, tag="mfinal")
    for g in range(2):
        massf = p2psr.tile([2, 4], F32, tag="mr")
        nc.tensor.matmul(massf, w["mask2"], xg[g][:, :, 0:1])
        nc.scalar.copy(msl2[:, 4 * g:4 * g + 4, NSLOT - 1], massf)
        minvf = p2s.tile([2, 4], F32, tag="minv")
        nc.vector.reciprocal(minvf, massf)
        rbcf = p2psr.tile([2 * S, 4], F32, tag="rb")
        nc.tensor.matmul(rbcf, w["mask2T"], minvf)
        i0, i1 = bass.broadcast_tensor_aps(xg[g][:, :, :], rbcf[:, :, None])
        nc.vector.tensor_tensor(mfinal[:, 4 * g:4 * g + 4, :], i0, i1,
                                ALU.mult)
    return mfinal


# ======================================================================
# host side
# ======================================================================
_PROGRAM_CACHE = {}


def _get_program(has_tb3):
    key = ("nc", has_tb3)
    if key not in _PROGRAM_CACHE:
        _PROGRAM_CACHE[key] = build_program(has_tb3)
    return _PROGRAM_CACHE[key]


def host_prep(inp):
    f32 = np.float32
    bf = ml_dtypes.bfloat16
    p = {}
    tw1 = np.asarray(inp["tw1"], f32)
    p["tw1cT"] = np.ascontiguousarray(tw1[:, :C].T).astype(bf)
    p["tb1p"] = (np.asarray(inp["tb1"], f32) + tw1[:, C:].sum(1) / S).astype(f32)
    p["tw2T"] = np.ascontiguousarray(np.asarray(inp["tw2"], f32).T).astype(bf)
    p["tb2"] = np.asarray(inp["tb2"], f32)
    p["tw3T"] = np.ascontiguousarray(np.asarray(inp["tw3"], f32).T).astype(bf)
    p["tb3_bf"] = np.asarray(inp["tb3"], f32).astype(bf)
    p["fw1T_bf"] = np.ascontiguousarray(np.asarray(inp["fw1"], f32).T).astype(bf)
    p["fb1_"] = np.asarray(inp["fb1"], f32)
    p["fw2T_bf"] = np.ascontiguousarray(np.asarray(inp["fw2"], f32).T).astype(bf)
    p["fb2_"] = np.asarray(inp["fb2"], f32)
    lw = np.asarray(inp["lw"], f32)
    lb = np.asarray(inp["lb"], f32)
    p["mwT_bf"] = np.ascontiguousarray(np.asarray(inp["mw"], f32).T).astype(bf)
    p["mb_"] = np.asarray(inp["mb"], f32)
    p["lwT_bf"] = np.ascontiguousarray(lw.T).astype(bf)
    p["lb_neg"] = (-lb).astype(f32)
    p["lwsum"] = lw.sum(0).astype(bf)
    se = np.asarray(inp["state_emb"], f32)
    off_mean = se @ np.asarray(inp["mw"], f32).T
    off_lv = se @ lw.T
    E1 = np.exp(-off_lv)
    # -0.5 of the quadratic form folded into the L matrices
    p["L1"] = np.ascontiguousarray((-0.5) * E1.T)
    p["L2m"] = np.ascontiguousarray((-0.5) * (-2.0 * off_mean * E1).T)
    p["L3"] = np.ascontiguousarray((-0.5) * (off_mean**2 * E1).T)
    p["olv_bias"] = (
        -0.5 * (D * math.log(2.0 * math.pi) + off_lv.sum(1) + lb.sum())
    ).astype(f32)
    return p


def build_in_maps(inputs):
    p = host_prep(inputs)
    obs = np.asarray(inputs["observations"], np.float32)
    ctx = np.asarray(inputs["context"], np.float32).astype(ml_dtypes.bfloat16)

    in_maps = []
    for k in range(NCORES):
        t0, t1 = SLAB * k, SLAB * (k + 1)
        alpha = np.ones((S, NCHAIN), np.float32)
        if k == 0:
            alpha[:, 0:4] = 0.0   # sub-0 chains on core 0: step-0 = identity
        alpha2 = np.ones((2 * S, NCHAIN // 2), np.float32)
        alpha2[0:S, :] = alpha[:, 0:NCHAIN // 2]
        alpha2[S:2 * S, :] = alpha[:, NCHAIN // 2:]
        m2t = np.zeros((2, 2 * S), np.float32)
        m2t[0, 0:S] = 1.0
        m2t[1, S:2 * S] = 1.0
        m = {
            "obs_slab": np.ascontiguousarray(obs[:, t0:t1, :]),
            "ctx_slab": np.ascontiguousarray(ctx[:, t0:t1, :]),
            "alpha_blend": alpha,
            "alpha_blend2": alpha2,
            "mask2T_": m2t,
        }
        m.update(p)
        in_maps.append(m)
    return in_maps


def kernel(**inputs):
    has_tb3 = bool(np.any(np.asarray(inputs["tb3"]) != 0))
    nc = _get_program(has_tb3)
    in_maps = build_in_maps(inputs)
    res = run_bass_kernel_spmd(nc, in_maps, core_ids=list(range(NCORES)))
    return np.asarray(res.results[0]["ll_out"], np.float32)


if __name__ == "__main__":
    sys.path.insert(0, "/root/problem")
    import reference

    inp = {k: np.asarray(v) for k, v in reference.setup_inputs().items()}
    got = kernel(**inp)
    print("kernel:", got)
